# revision 4
# baseline (speedup 1.0000x reference)
# Neural CDE (RK4 3/8-rule scan) Trainium2 Bass kernel.
# Self-contained: hardcodes shapes/sharding; callable as kernel(**inputs).
import sys

sys.path.insert(0, "/opt/trn_rl_repo")
import numpy as np

B, T, IN, HID, HH, OUT = 1024, 128, 16, 64, 64, 10
NCORES = 8
BL = B // NCORES  # batch per core
STEPS = T - 1

_cache = {}


def _build(steps=STEPS):
    import concourse.bacc as bacc
    import concourse.mybir as mybir
    from concourse.tile import TileContext

    fp32 = mybir.dt.float32
    AF = mybir.ActivationFunctionType
    OP = mybir.AluOpType
    AX = mybir.AxisListType

    nc = bacc.Bacc("TRN2", target_bir_lowering=False, debug=False, num_devices=NCORES)

    # ---- DRAM I/O (per-core shards; weights replicated) ----
    d_a0 = nc.dram_tensor("a0", (BL, IN), fp32, kind="ExternalInput")
    d_cb = nc.dram_tensor("cb", (BL, STEPS * IN), fp32, kind="ExternalInput")
    d_cc = nc.dram_tensor("cc", (BL, STEPS * IN), fp32, kind="ExternalInput")
    d_cd = nc.dram_tensor("cd", (BL, STEPS * IN), fp32, kind="ExternalInput")
    d_w1 = nc.dram_tensor("w1aug", (HID + 1, HH), fp32, kind="ExternalInput")
    d_w2 = nc.dram_tensor("w2aug", (HH + 1, HID * IN), fp32, kind="ExternalInput")
    d_wl = nc.dram_tensor("wlaug", (HID + 1, OUT), fp32, kind="ExternalInput")
    d_wi = nc.dram_tensor("wiaug", (IN + 1, HID), fp32, kind="ExternalInput")
    d_id = nc.dram_tensor("ident", (128, 128), fp32, kind="ExternalInput")
    d_ones = nc.dram_tensor("ones", (1, 128), fp32, kind="ExternalInput")
    d_y = nc.dram_tensor("y", (BL, T * OUT), fp32, kind="ExternalOutput")

    NC2 = STEPS * IN  # 2032

    with TileContext(nc) as tc:
        with (
            tc.tile_pool(name="const", bufs=1) as constp,
            tc.tile_pool(name="state", bufs=1) as statep,
            tc.tile_pool(name="big", bufs=1) as bigp,
            tc.tile_pool(name="fpool", bufs=2) as fpool,
            tc.tile_pool(name="hpsum", bufs=2, space="PSUM") as hpsum,
            tc.tile_pool(name="fpsum", bufs=1, space="PSUM") as fpsum,
            tc.tile_pool(name="tpsum", bufs=2, space="PSUM") as tpsum,
            tc.tile_pool(name="ypsum", bufs=1, space="PSUM") as ypsum,
        ):
            # ---- constants ----
            w1 = constp.tile([HID + 1, HH], fp32, tag="w1")
            w2 = constp.tile([HH + 1, HID * IN], fp32, tag="w2")
            wl = constp.tile([HID + 1, OUT], fp32, tag="wl")
            wi = constp.tile([IN + 1, HID], fp32, tag="wi")
            ident = constp.tile([128, 128], fp32, tag="ident")
            nc.sync.dma_start(w1[:], d_w1.ap()[:])
            nc.sync.dma_start(w2[:], d_w2.ap()[:])
            nc.sync.dma_start(wl[:], d_wl.ap()[:])
            nc.sync.dma_start(wi[:], d_wi.ap()[:])
            nc.sync.dma_start(ident[:], d_id.ap()[:])

            # ---- spline derivative precompute: u, v, w ----
            cb = bigp.tile([BL, NC2], fp32, tag="cb")
            cc = bigp.tile([BL, NC2], fp32, tag="cc")
            cd = bigp.tile([BL, NC2], fp32, tag="cd")
            uu = bigp.tile([BL, NC2], fp32, tag="uu")
            vv = bigp.tile([BL, NC2], fp32, tag="vv")
            ww = bigp.tile([BL, NC2], fp32, tag="ww")
            nc.sync.dma_start(cb[:], d_cb.ap()[:])
            nc.sync.dma_start(cc[:], d_cc.ap()[:])
            nc.sync.dma_start(cd[:], d_cd.ap()[:])
            stt = nc.vector.scalar_tensor_tensor
            # u = b + c + d ; v = b + c/3 + d/9 ; w = b + 2c/3 + 4d/9
            stt(uu[:], cc[:], 1.0, cb[:], OP.mult, OP.add)
            stt(uu[:], cd[:], 1.0, uu[:], OP.mult, OP.add)
            stt(vv[:], cc[:], 1.0 / 3.0, cb[:], OP.mult, OP.add)
            stt(vv[:], cd[:], 1.0 / 9.0, vv[:], OP.mult, OP.add)
            stt(ww[:], cc[:], 2.0 / 3.0, cb[:], OP.mult, OP.add)
            stt(ww[:], cd[:], 4.0 / 9.0, ww[:], OP.mult, OP.add)

            # ---- state tiles ----
            zT = statep.tile([HID + 1, BL], fp32, tag="zT")  # master state (T-layout, aug)
            zmid = statep.tile([HID + 1, BL], fp32, tag="zmid")
            hT = statep.tile([HH + 1, BL], fp32, tag="hT")
            k1 = statep.tile([HID, BL], fp32, tag="k1")
            k2 = statep.tile([HID, BL], fp32, tag="k2")
            k3 = statep.tile([HID, BL], fp32, tag="k3")
            tA = statep.tile([HID, BL], fp32, tag="tA")
            tB = statep.tile([HID, BL], fp32, tag="tB")
            y_sb = statep.tile([BL, T * OUT], fp32, tag="y_sb")
            nc.sync.dma_start(zT[HID : HID + 1, :], d_ones.ap()[:])
            nc.sync.dma_start(zmid[HID : HID + 1, :], d_ones.ap()[:])
            nc.sync.dma_start(hT[HH : HH + 1, :], d_ones.ap()[:])

            # ---- z0 = a0 @ W_init + b_init ----
            with tc.tile_pool(name="prol", bufs=1) as prol:
                a0 = prol.tile([BL, IN], fp32, tag="a0")
                a0T = prol.tile([IN + 1, BL], fp32, tag="a0T")
                nc.sync.dma_start(a0[:], d_a0.ap()[:])
                a0Tp = tpsum.tile([IN, BL], fp32, tag="ftp")
                nc.tensor.transpose(a0Tp[:], a0[:], ident[:])
                nc.vector.tensor_copy(a0T[0:IN, :], a0Tp[:])
                nc.sync.dma_start(a0T[IN : IN + 1, :], d_ones.ap()[:])
                z0p = tpsum.tile([HID, BL], fp32, tag="ftp")
                nc.tensor.matmul(z0p[:], wi[:], a0T[:], start=True, stop=True)
                nc.vector.tensor_copy(zT[0:HID, :], z0p[:])

            # ---- y for t=0 ----
            yp = ypsum.tile([BL, OUT], fp32, tag="yp")
            nc.tensor.matmul(yp[:], zT[:], wl[:], start=True, stop=True)
            nc.scalar.activation(y_sb[:, 0:OUT], yp[:], AF.Copy)

            kk = (k1, k2, k3, None)

            # ---- main RK4 scan ----
            for t in range(steps):
                for s in range(4):
                    src = zT if s == 0 else zmid
                    # mm1 + relu
                    hp = hpsum.tile([HH, BL], fp32, tag="hp")
                    nc.tensor.matmul(hp[:], w1[:], src[:], start=True, stop=True)
                    nc.scalar.activation(hT[0:HH, :], hp[:], AF.Relu)
                    # mm2 + tanh
                    fp = fpsum.tile([BL, HID * IN], fp32, tag="fp")
                    nc.tensor.matmul(fp[:, 0:512], hT[:], w2[:, 0:512], start=True, stop=True)
                    nc.tensor.matmul(fp[:, 512:1024], hT[:], w2[:, 512:1024], start=True, stop=True)
                    f_sb = fpool.tile([BL, HID * IN], fp32, tag="f_sb")
                    nc.scalar.activation(f_sb[:], fp[:], AF.Tanh)
                    # dX slice for this stage
                    if s == 0:
                        dx = cb[:, 0:IN] if t == 0 else uu[:, (t - 1) * IN : t * IN]
                    elif s == 1:
                        dx = vv[:, t * IN : (t + 1) * IN]
                    elif s == 2:
                        dx = ww[:, t * IN : (t + 1) * IN]
                    else:
                        dx = uu[:, t * IN : (t + 1) * IN]
                    # g = f * dX (broadcast over h), F = sum_i g
                    g = fpool.tile([BL, HID * IN], fp32, tag="g")
                    g3 = g.rearrange("p (h i) -> p h i", i=IN)
                    f3 = f_sb.rearrange("p (h i) -> p h i", i=IN)
                    dx3 = dx.unsqueeze(1).broadcast_to([BL, HID, IN])
                    nc.vector.tensor_tensor(g3, f3, dx3, OP.mult)
                    F = fpool.tile([BL, HID], fp32, tag="F")
                    nc.vector.tensor_reduce(F[:], g3, AX.X, OP.add)
                    # F^T via PE transpose
                    ftp = tpsum.tile([HID, BL], fp32, tag="ftp")
                    nc.tensor.transpose(ftp[:], F[:], ident[:])
                    # combos
                    if s < 3:
                        nc.vector.tensor_copy(kk[s][:], ftp[:])
                    if s == 0:
                        # za = z + k1/3
                        stt(zmid[0:HID, :], k1[:], 1.0 / 3.0, zT[0:HID, :], OP.mult, OP.add)
                    elif s == 1:
                        # zb = z - k1/3 + k2
                        stt(tA[:], k1[:], -1.0 / 3.0, zT[0:HID, :], OP.mult, OP.add)
                        stt(zmid[0:HID, :], k2[:], 1.0, tA[:], OP.mult, OP.add)
                    elif s == 2:
                        # zc = z + k1 - k2 + k3
                        stt(tA[:], k1[:], 1.0, zT[0:HID, :], OP.mult, OP.add)
                        stt(tB[:], k2[:], -1.0, tA[:], OP.mult, OP.add)
                        stt(zmid[0:HID, :], k3[:], 1.0, tB[:], OP.mult, OP.add)
                    else:
                        # zn = z + (k1 + 3(k2+k3) + k4)/8
                        stt(tA[:], k2[:], 1.0, k3[:], OP.mult, OP.add)
                        stt(tB[:], tA[:], 3.0, k1[:], OP.mult, OP.add)
                        stt(tA[:], ftp[:], 1.0, tB[:], OP.mult, OP.add)
                        stt(zT[0:HID, :], tA[:], 0.125, zT[0:HID, :], OP.mult, OP.add)
                # y_{t+1} = z @ W_lin + b_lin
                yp = ypsum.tile([BL, OUT], fp32, tag="yp")
                nc.tensor.matmul(yp[:], zT[:], wl[:], start=True, stop=True)
                nc.scalar.activation(y_sb[:, (t + 1) * OUT : (t + 2) * OUT], yp[:], AF.Copy)

            nc.sync.dma_start(d_y.ap()[:], y_sb[:])

    nc.compile()
    return nc


def _prep_maps(coeff_a, coeff_b, coeff_two_c, coeff_three_d, W_init, b_init, W1, b1, W2, b2, W_lin, b_lin):
    f32 = np.float32
    w1aug = np.concatenate([np.asarray(W1, f32), np.asarray(b1, f32)[None]], 0)
    w2aug = np.concatenate([np.asarray(W2, f32), np.asarray(b2, f32)[None]], 0)
    wlaug = np.concatenate([np.asarray(W_lin, f32), np.asarray(b_lin, f32)[None]], 0)
    wiaug = np.concatenate([np.asarray(W_init, f32), np.asarray(b_init, f32)[None]], 0)
    ident = np.eye(128, dtype=f32)
    ca = np.asarray(coeff_a, f32)
    cb = np.asarray(coeff_b, f32)
    cc = np.asarray(coeff_two_c, f32)
    cd = np.asarray(coeff_three_d, f32)
    maps = []
    for c in range(NCORES):
        sl = slice(c * BL, (c + 1) * BL)
        maps.append(
            {
                "a0": np.ascontiguousarray(ca[sl, 0, :]),
                "cb": np.ascontiguousarray(cb[sl].reshape(BL, -1)),
                "cc": np.ascontiguousarray(cc[sl].reshape(BL, -1)),
                "cd": np.ascontiguousarray(cd[sl].reshape(BL, -1)),
                "w1aug": w1aug,
                "w2aug": w2aug,
                "wlaug": wlaug,
                "wiaug": wiaug,
                "ident": ident,
                "ones": np.ones((1, 128), f32),
            }
        )
    return maps


def _install_ntff_hook():
    """Provide antenv.axon_hooks (missing in this image) so trace=True works."""
    import types, ctypes, contextlib

    try:
        from antenv.axon_hooks import get_axon_ntff_profile_hook  # noqa: F401
        return
    except ImportError:
        pass
    import antenv

    hook = None
    try:
        lib = ctypes.CDLL("/opt/axon/libaxon_pjrt.so")
        if hasattr(lib, "axon_start_nrt_profile"):
            lib.axon_start_nrt_profile.argtypes = [
                ctypes.POINTER(ctypes.c_int64),
                ctypes.c_size_t,
            ]
            lib.axon_start_nrt_profile.restype = ctypes.c_int64
            lib.axon_stop_nrt_profile.argtypes = [ctypes.c_char_p]
            lib.axon_stop_nrt_profile.restype = ctypes.c_int64

            @contextlib.contextmanager
            def _hook(output_dir, device_ids):
                import jax

                jax.devices()
                if device_ids:
                    ids = (ctypes.c_int64 * len(device_ids))(*device_ids)
                    rc = lib.axon_start_nrt_profile(ids, len(device_ids))
                else:
                    rc = lib.axon_start_nrt_profile(None, 0)
                if rc != 0:
                    raise RuntimeError(f"axon_start_nrt_profile rc={rc}")
                try:
                    yield
                finally:
                    n = lib.axon_stop_nrt_profile(str(output_dir).encode())
                    print(f"ntff profile: {n} file(s) written to {output_dir}")

            hook = _hook
    except OSError:
        pass
    mod = types.ModuleType("antenv.axon_hooks")
    mod.get_axon_ntff_profile_hook = lambda: hook
    mod.set_axon_ntff_profile_hook = lambda h: None
    sys.modules["antenv.axon_hooks"] = mod
    antenv.axon_hooks = mod


def kernel(times, coeff_a, coeff_b, coeff_two_c, coeff_three_d, final_index,
           W_init, b_init, W1, b1, W2, b2, W_lin, b_lin, trace=False):
    from concourse import bass_utils

    if trace:
        _install_ntff_hook()

    if "nc" not in _cache:
        _cache["nc"] = _build()
    nc = _cache["nc"]
    maps = _prep_maps(coeff_a, coeff_b, coeff_two_c, coeff_three_d,
                      W_init, b_init, W1, b1, W2, b2, W_lin, b_lin)
    res = bass_utils.run_bass_kernel_spmd(nc, maps, core_ids=list(range(NCORES)), trace=trace)
    _cache["last_res"] = res
    y = np.concatenate(
        [res.results[c]["y"].reshape(BL, T, OUT) for c in range(NCORES)], 0
    )
    return y.astype(np.float32)


# revision 6
# speedup vs baseline: 1.6211x; 1.6211x over previous
# Neural CDE (RK4 3/8-rule scan) Trainium2 Bass kernel.
# Self-contained: hardcodes shapes/sharding; callable as kernel(**inputs).
import sys

sys.path.insert(0, "/opt/trn_rl_repo")
import numpy as np

B, T, IN, HID, HH, OUT = 1024, 128, 16, 64, 64, 10
NCORES = 8
BL = B // NCORES  # batch per core
STEPS = T - 1

_cache = {}


def _build(steps=STEPS, use_f32r=True, half_dt="float16"):
    import concourse.bacc as bacc
    import concourse.mybir as mybir
    from concourse.tile import TileContext

    fp32 = mybir.dt.float32
    f32r = mybir.dt.float32r if use_f32r else fp32
    fp16 = getattr(mybir.dt, half_dt)
    AF = mybir.ActivationFunctionType
    OP = mybir.AluOpType
    AX = mybir.AxisListType

    nc = bacc.Bacc("TRN2", target_bir_lowering=False, debug=False, num_devices=NCORES)

    # ---- DRAM I/O (per-core shards; weights replicated) ----
    d_a0 = nc.dram_tensor("a0", (BL, IN), fp32, kind="ExternalInput")
    d_cb = nc.dram_tensor("cb", (BL, STEPS * IN), fp32, kind="ExternalInput")
    d_cc = nc.dram_tensor("cc", (BL, STEPS * IN), fp32, kind="ExternalInput")
    d_cd = nc.dram_tensor("cd", (BL, STEPS * IN), fp32, kind="ExternalInput")
    d_w1 = nc.dram_tensor("w1aug", (HID + 1, HH), fp32, kind="ExternalInput")
    d_w2 = nc.dram_tensor("w2aug", (HH + 1, HID * IN), fp32, kind="ExternalInput")
    d_wl = nc.dram_tensor("wlaug", (HID + 1, OUT), fp32, kind="ExternalInput")
    d_wi = nc.dram_tensor("wiaug", (IN + 1, HID), fp32, kind="ExternalInput")
    d_id = nc.dram_tensor("ident", (128, 128), fp32, kind="ExternalInput")
    d_idh = nc.dram_tensor("identh", (128, 128), fp16, kind="ExternalInput")
    d_ones = nc.dram_tensor("ones", (1, 128), fp32, kind="ExternalInput")
    d_onesr = nc.dram_tensor("onesr", (1, 128), f32r, kind="ExternalInput")
    d_y = nc.dram_tensor("y", (BL, T * OUT), fp32, kind="ExternalOutput")

    NC2 = STEPS * IN  # 2032
    H2 = HID // 2  # 32 (column-split half of h)

    with TileContext(nc) as tc, nc.allow_low_precision("fp16 f-path; fp32 state"):
        with (
            tc.tile_pool(name="const", bufs=1) as constp,
            tc.tile_pool(name="state", bufs=1) as statep,
            tc.tile_pool(name="big", bufs=1) as bigp,
            tc.tile_pool(name="fpool", bufs=3) as fpool,
            tc.tile_pool(name="hpsum", bufs=2, space="PSUM") as hpsum,
            tc.tile_pool(name="fpsum", bufs=2, space="PSUM") as fpsum,
            tc.tile_pool(name="tpsum", bufs=2, space="PSUM") as tpsum,
            tc.tile_pool(name="ypsum", bufs=1, space="PSUM") as ypsum,
        ):
            # ---- constants ----
            w1 = constp.tile([HID + 1, HH], fp32, tag="w1")
            w2 = constp.tile([HH + 1, HID * IN], fp32, tag="w2")
            w2r = constp.tile([HH + 1, HID * IN], f32r, tag="w2r")
            wl = constp.tile([HID + 1, OUT], fp32, tag="wl")
            wi = constp.tile([IN + 1, HID], fp32, tag="wi")
            ident = constp.tile([128, 128], fp32, tag="ident")
            identh = constp.tile([128, 128], fp16, tag="identh")
            nc.sync.dma_start(w1[:], d_w1.ap()[:])
            nc.sync.dma_start(w2[:], d_w2.ap()[:])
            nc.sync.dma_start(wl[:], d_wl.ap()[:])
            nc.sync.dma_start(wi[:], d_wi.ap()[:])
            nc.sync.dma_start(ident[:], d_id.ap()[:])
            nc.sync.dma_start(identh[:], d_idh.ap()[:])
            nc.vector.tensor_copy(w2r[:], w2[:])

            # ---- spline derivative precompute: u, v, w (fp16) ----
            cb = bigp.tile([BL, NC2], fp32, tag="cb")
            cc = bigp.tile([BL, NC2], fp32, tag="cc")
            cd = bigp.tile([BL, NC2], fp32, tag="cd")
            tt0 = bigp.tile([BL, NC2], fp32, tag="tt0")
            uu = bigp.tile([BL, NC2], fp16, tag="uu")
            vv = bigp.tile([BL, NC2], fp16, tag="vv")
            ww = bigp.tile([BL, NC2], fp16, tag="ww")
            db0 = bigp.tile([BL, IN], fp16, tag="db0")
            nc.sync.dma_start(cb[:], d_cb.ap()[:])
            nc.sync.dma_start(cc[:], d_cc.ap()[:])
            nc.sync.dma_start(cd[:], d_cd.ap()[:])
            stt = nc.vector.scalar_tensor_tensor
            # u = b + c + d ; v = b + c/3 + d/9 ; w = b + 2c/3 + 4d/9
            stt(tt0[:], cc[:], 1.0, cb[:], OP.mult, OP.add)
            stt(uu[:], cd[:], 1.0, tt0[:], OP.mult, OP.add)
            stt(tt0[:], cc[:], 1.0 / 3.0, cb[:], OP.mult, OP.add)
            stt(vv[:], cd[:], 1.0 / 9.0, tt0[:], OP.mult, OP.add)
            stt(tt0[:], cc[:], 2.0 / 3.0, cb[:], OP.mult, OP.add)
            stt(ww[:], cd[:], 4.0 / 9.0, tt0[:], OP.mult, OP.add)
            nc.vector.tensor_copy(db0[:], cb[:, 0:IN])

            # ---- state tiles ----
            zT = statep.tile([HID + 1, BL], fp32, tag="zT")  # master state (T-layout, aug)
            zmid = statep.tile([HID + 1, BL], fp32, tag="zmid")
            hT = statep.tile([HH + 1, BL], f32r, tag="hT")
            k1 = statep.tile([HID, BL], fp32, tag="k1")
            k2 = statep.tile([HID, BL], fp32, tag="k2")
            k3 = statep.tile([HID, BL], fp32, tag="k3")
            tA = statep.tile([HID, BL], fp32, tag="tA")
            tB = statep.tile([HID, BL], fp32, tag="tB")
            y_sb = statep.tile([BL, T * OUT], fp32, tag="y_sb")
            nc.sync.dma_start(zT[HID : HID + 1, :], d_ones.ap()[:])
            nc.sync.dma_start(zmid[HID : HID + 1, :], d_ones.ap()[:])
            nc.sync.dma_start(hT[HH : HH + 1, :], d_onesr.ap()[:])

            # ---- z0 = a0 @ W_init + b_init ----
            with tc.tile_pool(name="prol", bufs=1) as prol:
                a0 = prol.tile([BL, IN], fp32, tag="a0")
                a0T = prol.tile([IN + 1, BL], fp32, tag="a0T")
                nc.sync.dma_start(a0[:], d_a0.ap()[:])
                a0Tp = tpsum.tile([IN, BL], fp32, tag="ftp")
                nc.tensor.transpose(a0Tp[:], a0[:], ident[:])
                nc.vector.tensor_copy(a0T[0:IN, :], a0Tp[:])
                nc.sync.dma_start(a0T[IN : IN + 1, :], d_ones.ap()[:])
                z0p = tpsum.tile([HID, BL], fp32, tag="ftp")
                nc.tensor.matmul(z0p[:], wi[:], a0T[:], start=True, stop=True)
                nc.vector.tensor_copy(zT[0:HID, :], z0p[:])

            # ---- y for t=0 ----
            yp = ypsum.tile([BL, OUT], fp32, tag="yp")
            nc.tensor.matmul(yp[:], zT[:], wl[:], start=True, stop=True)
            nc.scalar.activation(y_sb[:, 0:OUT], yp[:], AF.Copy)

            kk = (k1, k2, k3, None)

            # ---- main RK4 scan ----
            for t in range(steps):
                for s in range(4):
                    src = zT if s == 0 else zmid
                    # mm1 (fp32) + relu (-> f32r for the mm2 weights-side)
                    hp = hpsum.tile([HH, BL], fp32, tag="hp")
                    nc.tensor.matmul(hp[:], w1[:], src[:], start=True, stop=True)
                    nc.scalar.activation(hT[0:HH, :], hp[:], AF.Relu)
                    # dX slice for this stage
                    if s == 0:
                        dx = db0[:] if t == 0 else uu[:, (t - 1) * IN : t * IN]
                    elif s == 1:
                        dx = vv[:, t * IN : (t + 1) * IN]
                    elif s == 2:
                        dx = ww[:, t * IN : (t + 1) * IN]
                    else:
                        dx = uu[:, t * IN : (t + 1) * IN]
                    dx3 = dx.unsqueeze(1).broadcast_to([BL, H2, IN])
                    F = fpool.tile([BL, HID], fp16, tag="F")
                    ftp = tpsum.tile([HID, BL], fp16, tag="ftp")
                    # column-split halves pipeline mm2->tanh->mult->reduce->transpose
                    for ch in range(2):
                        cols = slice(ch * 512, (ch + 1) * 512)
                        fpp = fpsum.tile([BL, 512], fp32, tag="fp")
                        nc.tensor.matmul(fpp[:], hT[:], w2r[:, cols], start=True, stop=True)
                        f_sb = fpool.tile([BL, 512], fp16, tag="f_sb")
                        nc.scalar.activation(f_sb[:], fpp[:], AF.Tanh)
                        g = fpool.tile([BL, 512], fp16, tag="g")
                        g3 = g.rearrange("p (h i) -> p h i", i=IN)
                        f3 = f_sb.rearrange("p (h i) -> p h i", i=IN)
                        nc.vector.tensor_tensor(g3, f3, dx3, OP.mult)
                        nc.vector.tensor_reduce(
                            F[:, ch * H2 : (ch + 1) * H2], g3, AX.X, OP.add
                        )
                        nc.tensor.transpose(
                            ftp[ch * H2 : (ch + 1) * H2, :],
                            F[:, ch * H2 : (ch + 1) * H2],
                            identh[:],
                        )
                    # combos (fp32 state updates)
                    if s < 3:
                        nc.vector.tensor_copy(kk[s][:], ftp[:])
                    if s == 0:
                        # za = z + k1/3
                        stt(zmid[0:HID, :], k1[:], 1.0 / 3.0, zT[0:HID, :], OP.mult, OP.add)
                    elif s == 1:
                        # zb = z - k1/3 + k2
                        stt(tA[:], k1[:], -1.0 / 3.0, zT[0:HID, :], OP.mult, OP.add)
                        stt(zmid[0:HID, :], k2[:], 1.0, tA[:], OP.mult, OP.add)
                    elif s == 2:
                        # zc = z + k1 - k2 + k3
                        stt(tA[:], k1[:], 1.0, zT[0:HID, :], OP.mult, OP.add)
                        stt(tB[:], k2[:], -1.0, tA[:], OP.mult, OP.add)
                        stt(zmid[0:HID, :], k3[:], 1.0, tB[:], OP.mult, OP.add)
                    else:
                        # zn = z + (k1 + 3(k2+k3) + k4)/8
                        stt(tA[:], k2[:], 1.0, k3[:], OP.mult, OP.add)
                        stt(tB[:], tA[:], 3.0, k1[:], OP.mult, OP.add)
                        stt(tA[:], ftp[:], 1.0, tB[:], OP.mult, OP.add)
                        stt(zT[0:HID, :], tA[:], 0.125, zT[0:HID, :], OP.mult, OP.add)
                # y_{t+1} = z @ W_lin + b_lin
                yp = ypsum.tile([BL, OUT], fp32, tag="yp")
                nc.tensor.matmul(yp[:], zT[:], wl[:], start=True, stop=True)
                nc.scalar.activation(y_sb[:, (t + 1) * OUT : (t + 2) * OUT], yp[:], AF.Copy)

            nc.sync.dma_start(d_y.ap()[:], y_sb[:])

    nc.compile()
    return nc


def _prep_maps(coeff_a, coeff_b, coeff_two_c, coeff_three_d, W_init, b_init, W1, b1, W2, b2, W_lin, b_lin):
    import ml_dtypes

    f32 = np.float32
    f16 = np.float16
    w1aug = np.concatenate([np.asarray(W1, f32), np.asarray(b1, f32)[None]], 0)
    w2aug = np.concatenate([np.asarray(W2, f32), np.asarray(b2, f32)[None]], 0)
    wlaug = np.concatenate([np.asarray(W_lin, f32), np.asarray(b_lin, f32)[None]], 0)
    wiaug = np.concatenate([np.asarray(W_init, f32), np.asarray(b_init, f32)[None]], 0)
    ident = np.eye(128, dtype=f32)
    ca = np.asarray(coeff_a, f32)
    cb = np.asarray(coeff_b, f32)
    cc = np.asarray(coeff_two_c, f32)
    cd = np.asarray(coeff_three_d, f32)
    maps = []
    for c in range(NCORES):
        sl = slice(c * BL, (c + 1) * BL)
        maps.append(
            {
                "a0": np.ascontiguousarray(ca[sl, 0, :]),
                "cb": np.ascontiguousarray(cb[sl].reshape(BL, -1)),
                "cc": np.ascontiguousarray(cc[sl].reshape(BL, -1)),
                "cd": np.ascontiguousarray(cd[sl].reshape(BL, -1)),
                "w1aug": w1aug,
                "w2aug": w2aug,
                "wlaug": wlaug,
                "wiaug": wiaug,
                "ident": ident,
                "identh": np.eye(128, dtype=f16),
                "ones": np.ones((1, 128), f32),
                "onesr": np.ones((1, 128), f32),
            }
        )
    return maps


def _install_ntff_hook():
    """Provide antenv.axon_hooks (missing in this image) so trace=True works."""
    import types, ctypes, contextlib

    try:
        from antenv.axon_hooks import get_axon_ntff_profile_hook  # noqa: F401
        return
    except ImportError:
        pass
    import antenv

    hook = None
    try:
        lib = ctypes.CDLL("/opt/axon/libaxon_pjrt.so")
        if hasattr(lib, "axon_start_nrt_profile"):
            lib.axon_start_nrt_profile.argtypes = [
                ctypes.POINTER(ctypes.c_int64),
                ctypes.c_size_t,
            ]
            lib.axon_start_nrt_profile.restype = ctypes.c_int64
            lib.axon_stop_nrt_profile.argtypes = [ctypes.c_char_p]
            lib.axon_stop_nrt_profile.restype = ctypes.c_int64

            @contextlib.contextmanager
            def _hook(output_dir, device_ids):
                import jax

                jax.devices()
                if device_ids:
                    ids = (ctypes.c_int64 * len(device_ids))(*device_ids)
                    rc = lib.axon_start_nrt_profile(ids, len(device_ids))
                else:
                    rc = lib.axon_start_nrt_profile(None, 0)
                if rc != 0:
                    raise RuntimeError(f"axon_start_nrt_profile rc={rc}")
                try:
                    yield
                finally:
                    n = lib.axon_stop_nrt_profile(str(output_dir).encode())
                    print(f"ntff profile: {n} file(s) written to {output_dir}")

            hook = _hook
    except OSError:
        pass
    mod = types.ModuleType("antenv.axon_hooks")
    mod.get_axon_ntff_profile_hook = lambda: hook
    mod.set_axon_ntff_profile_hook = lambda h: None
    sys.modules["antenv.axon_hooks"] = mod
    antenv.axon_hooks = mod


def kernel(times, coeff_a, coeff_b, coeff_two_c, coeff_three_d, final_index,
           W_init, b_init, W1, b1, W2, b2, W_lin, b_lin, trace=False):
    from concourse import bass_utils

    if trace:
        _install_ntff_hook()

    if "nc" not in _cache:
        _cache["nc"] = _build()
    nc = _cache["nc"]
    maps = _prep_maps(coeff_a, coeff_b, coeff_two_c, coeff_three_d,
                      W_init, b_init, W1, b1, W2, b2, W_lin, b_lin)
    res = bass_utils.run_bass_kernel_spmd(nc, maps, core_ids=list(range(NCORES)), trace=trace)
    _cache["last_res"] = res
    y = np.concatenate(
        [res.results[c]["y"].reshape(BL, T, OUT) for c in range(NCORES)], 0
    )
    return y.astype(np.float32)


# revision 7
# speedup vs baseline: 1.8221x; 1.1240x over previous
# Neural CDE (RK4 3/8-rule scan) Trainium2 Bass kernel.
# Self-contained: hardcodes shapes/sharding; callable as kernel(**inputs).
import sys

sys.path.insert(0, "/opt/trn_rl_repo")
import numpy as np

B, T, IN, HID, HH, OUT = 1024, 128, 16, 64, 64, 10
NCORES = 8
BL = B // NCORES  # batch per core
STEPS = T - 1

_cache = {}


def _build(steps=STEPS, use_f32r=True, half_dt="float16"):
    import concourse.bacc as bacc
    import concourse.mybir as mybir
    from concourse.tile import TileContext

    fp32 = mybir.dt.float32
    f32r = mybir.dt.float32r if use_f32r else fp32
    fp16 = getattr(mybir.dt, half_dt)
    AF = mybir.ActivationFunctionType
    OP = mybir.AluOpType
    AX = mybir.AxisListType

    nc = bacc.Bacc("TRN2", target_bir_lowering=False, debug=False, num_devices=NCORES)

    # ---- DRAM I/O (per-core shards; weights replicated) ----
    d_a0 = nc.dram_tensor("a0", (BL, IN), fp32, kind="ExternalInput")
    d_cb = nc.dram_tensor("cb", (BL, STEPS * IN), fp32, kind="ExternalInput")
    d_cc = nc.dram_tensor("cc", (BL, STEPS * IN), fp32, kind="ExternalInput")
    d_cd = nc.dram_tensor("cd", (BL, STEPS * IN), fp32, kind="ExternalInput")
    d_w1 = nc.dram_tensor("w1aug", (HID + 1, HH), fp16, kind="ExternalInput")
    d_w2 = nc.dram_tensor("w2aug", (HH + 1, HID * IN), fp16, kind="ExternalInput")
    d_wl = nc.dram_tensor("wlaug", (HID + 1, OUT), fp32, kind="ExternalInput")
    d_wi = nc.dram_tensor("wiaug", (IN + 1, HID), fp32, kind="ExternalInput")
    d_id = nc.dram_tensor("ident", (128, 128), fp32, kind="ExternalInput")
    d_idh = nc.dram_tensor("identh", (128, 128), fp16, kind="ExternalInput")
    d_ones = nc.dram_tensor("ones", (1, 128), fp32, kind="ExternalInput")
    d_onesh = nc.dram_tensor("onesh", (1, 128), fp16, kind="ExternalInput")
    d_y = nc.dram_tensor("y", (BL, T * OUT), fp32, kind="ExternalOutput")

    NC2 = STEPS * IN  # 2032
    H2 = HID // 2  # 32 (column-split half of h)

    with TileContext(nc) as tc, nc.allow_low_precision("fp16 f-path; fp32 state"):
        with (
            tc.tile_pool(name="const", bufs=1) as constp,
            tc.tile_pool(name="state", bufs=1) as statep,
            tc.tile_pool(name="big", bufs=1) as bigp,
            tc.tile_pool(name="fpool", bufs=3) as fpool,
            tc.tile_pool(name="hpsum", bufs=2, space="PSUM") as hpsum,
            tc.tile_pool(name="fpsum", bufs=2, space="PSUM") as fpsum,
            tc.tile_pool(name="tpsum", bufs=2, space="PSUM") as tpsum,
            tc.tile_pool(name="ypsum", bufs=1, space="PSUM") as ypsum,
        ):
            # ---- constants ----
            w1 = constp.tile([HID + 1, HH], fp16, tag="w1")
            w2 = constp.tile([HH + 1, HID * IN], fp16, tag="w2")
            wl = constp.tile([HID + 1, OUT], fp32, tag="wl")
            wi = constp.tile([IN + 1, HID], fp32, tag="wi")
            ident = constp.tile([128, 128], fp32, tag="ident")
            identh = constp.tile([128, 128], fp16, tag="identh")
            nc.sync.dma_start(w1[:], d_w1.ap()[:])
            nc.sync.dma_start(w2[:], d_w2.ap()[:])
            nc.sync.dma_start(wl[:], d_wl.ap()[:])
            nc.sync.dma_start(wi[:], d_wi.ap()[:])
            nc.sync.dma_start(ident[:], d_id.ap()[:])
            nc.sync.dma_start(identh[:], d_idh.ap()[:])

            # ---- spline derivative precompute: u, v, w (fp16) ----
            cb = bigp.tile([BL, NC2], fp32, tag="cb")
            cc = bigp.tile([BL, NC2], fp32, tag="cc")
            cd = bigp.tile([BL, NC2], fp32, tag="cd")
            tt0 = bigp.tile([BL, NC2], fp32, tag="tt0")
            uu = bigp.tile([BL, NC2], fp16, tag="uu")
            vv = bigp.tile([BL, NC2], fp16, tag="vv")
            ww = bigp.tile([BL, NC2], fp16, tag="ww")
            db0 = bigp.tile([BL, IN], fp16, tag="db0")
            nc.sync.dma_start(cb[:], d_cb.ap()[:])
            nc.sync.dma_start(cc[:], d_cc.ap()[:])
            nc.sync.dma_start(cd[:], d_cd.ap()[:])
            stt = nc.vector.scalar_tensor_tensor
            # u = b + c + d ; v = b + c/3 + d/9 ; w = b + 2c/3 + 4d/9
            stt(tt0[:], cc[:], 1.0, cb[:], OP.mult, OP.add)
            stt(uu[:], cd[:], 1.0, tt0[:], OP.mult, OP.add)
            stt(tt0[:], cc[:], 1.0 / 3.0, cb[:], OP.mult, OP.add)
            stt(vv[:], cd[:], 1.0 / 9.0, tt0[:], OP.mult, OP.add)
            stt(tt0[:], cc[:], 2.0 / 3.0, cb[:], OP.mult, OP.add)
            stt(ww[:], cd[:], 4.0 / 9.0, tt0[:], OP.mult, OP.add)
            nc.vector.tensor_copy(db0[:], cb[:, 0:IN])

            # ---- state tiles ----
            zT = statep.tile([HID + 1, BL], fp32, tag="zT")  # master state (T-layout, aug)
            zmid = statep.tile([HID + 1, BL], fp16, tag="zmid")
            zTh = statep.tile([HID + 1, BL], fp16, tag="zTh")
            hT = statep.tile([HH + 1, BL], fp16, tag="hT")
            k1 = statep.tile([HID, BL], fp32, tag="k1")
            k2 = statep.tile([HID, BL], fp32, tag="k2")
            k3 = statep.tile([HID, BL], fp32, tag="k3")
            tA = statep.tile([HID, BL], fp32, tag="tA")
            tB = statep.tile([HID, BL], fp32, tag="tB")
            y_sb = statep.tile([BL, T * OUT], fp32, tag="y_sb")
            nc.sync.dma_start(zT[HID : HID + 1, :], d_ones.ap()[:])
            nc.sync.dma_start(zmid[HID : HID + 1, :], d_onesh.ap()[:])
            nc.sync.dma_start(zTh[HID : HID + 1, :], d_onesh.ap()[:])
            nc.sync.dma_start(hT[HH : HH + 1, :], d_onesh.ap()[:])

            # ---- z0 = a0 @ W_init + b_init ----
            with tc.tile_pool(name="prol", bufs=1) as prol:
                a0 = prol.tile([BL, IN], fp32, tag="a0")
                a0T = prol.tile([IN + 1, BL], fp32, tag="a0T")
                nc.sync.dma_start(a0[:], d_a0.ap()[:])
                a0Tp = tpsum.tile([IN, BL], fp32, tag="ftp")
                nc.tensor.transpose(a0Tp[:], a0[:], ident[:])
                nc.vector.tensor_copy(a0T[0:IN, :], a0Tp[:])
                nc.sync.dma_start(a0T[IN : IN + 1, :], d_ones.ap()[:])
                z0p = tpsum.tile([HID, BL], fp32, tag="ftp")
                nc.tensor.matmul(z0p[:], wi[:], a0T[:], start=True, stop=True)
                nc.vector.tensor_copy(zT[0:HID, :], z0p[:])
                nc.vector.tensor_copy(zTh[0:HID, :], z0p[:])

            # ---- y for t=0 ----
            yp = ypsum.tile([BL, OUT], fp32, tag="yp")
            nc.tensor.matmul(yp[:], zT[:], wl[:], start=True, stop=True)
            nc.scalar.activation(y_sb[:, 0:OUT], yp[:], AF.Copy)

            kk = (k1, k2, k3, None)

            # ---- main RK4 scan ----
            for t in range(steps):
                for s in range(4):
                    src = zTh if s == 0 else zmid
                    # mm1 (fp32) + relu (-> f32r for the mm2 weights-side)
                    hp = hpsum.tile([HH, BL], fp32, tag="hp")
                    nc.tensor.matmul(hp[:], w1[:], src[:], start=True, stop=True)
                    nc.scalar.activation(hT[0:HH, :], hp[:], AF.Relu)
                    # dX slice for this stage
                    if s == 0:
                        dx = db0[:] if t == 0 else uu[:, (t - 1) * IN : t * IN]
                    elif s == 1:
                        dx = vv[:, t * IN : (t + 1) * IN]
                    elif s == 2:
                        dx = ww[:, t * IN : (t + 1) * IN]
                    else:
                        dx = uu[:, t * IN : (t + 1) * IN]
                    dx3 = dx.unsqueeze(1).broadcast_to([BL, H2, IN])
                    F = fpool.tile([BL, HID], fp16, tag="F")
                    ftp = tpsum.tile([HID, BL], fp16, tag="ftp")
                    # column-split halves pipeline mm2->tanh->mult->reduce->transpose
                    for ch in range(2):
                        cols = slice(ch * 512, (ch + 1) * 512)
                        fpp = fpsum.tile([BL, 512], fp32, tag="fp")
                        nc.tensor.matmul(fpp[:], hT[:], w2[:, cols], start=True, stop=True)
                        f_sb = fpool.tile([BL, 512], fp16, tag="f_sb")
                        nc.scalar.activation(f_sb[:], fpp[:], AF.Tanh)
                        g = fpool.tile([BL, 512], fp16, tag="g")
                        g3 = g.rearrange("p (h i) -> p h i", i=IN)
                        f3 = f_sb.rearrange("p (h i) -> p h i", i=IN)
                        nc.vector.tensor_tensor(g3, f3, dx3, OP.mult)
                        nc.vector.tensor_reduce(
                            F[:, ch * H2 : (ch + 1) * H2], g3, AX.X, OP.add
                        )
                        nc.tensor.transpose(
                            ftp[ch * H2 : (ch + 1) * H2, :],
                            F[:, ch * H2 : (ch + 1) * H2],
                            identh[:],
                        )
                    # combos (fp32 state updates)
                    if s < 3:
                        nc.scalar.activation(kk[s][:], ftp[:], AF.Copy)
                    if s == 0:
                        # za = z + k1/3
                        stt(zmid[0:HID, :], k1[:], 1.0 / 3.0, zT[0:HID, :], OP.mult, OP.add)
                    elif s == 1:
                        # zb = z - k1/3 + k2
                        stt(tA[:], k1[:], -1.0 / 3.0, zT[0:HID, :], OP.mult, OP.add)
                        stt(zmid[0:HID, :], k2[:], 1.0, tA[:], OP.mult, OP.add)
                    elif s == 2:
                        # zc = z + k1 - k2 + k3
                        stt(tA[:], k1[:], 1.0, zT[0:HID, :], OP.mult, OP.add)
                        stt(tB[:], k2[:], -1.0, tA[:], OP.mult, OP.add)
                        stt(zmid[0:HID, :], k3[:], 1.0, tB[:], OP.mult, OP.add)
                    else:
                        # zn = z + (k1 + 3(k2+k3) + k4)/8
                        stt(tA[:], k2[:], 1.0, k3[:], OP.mult, OP.add)
                        stt(tB[:], tA[:], 3.0, k1[:], OP.mult, OP.add)
                        stt(tA[:], ftp[:], 1.0, tB[:], OP.mult, OP.add)
                        stt(zTh[0:HID, :], tA[:], 0.125, zT[0:HID, :], OP.mult, OP.add)
                        stt(zT[0:HID, :], tA[:], 0.125, zT[0:HID, :], OP.mult, OP.add)
                # y_{t+1} = z @ W_lin + b_lin
                yp = ypsum.tile([BL, OUT], fp32, tag="yp")
                nc.tensor.matmul(yp[:], zT[:], wl[:], start=True, stop=True)
                nc.scalar.activation(y_sb[:, (t + 1) * OUT : (t + 2) * OUT], yp[:], AF.Copy)

            nc.sync.dma_start(d_y.ap()[:], y_sb[:])

    nc.compile()
    return nc


def _prep_maps(coeff_a, coeff_b, coeff_two_c, coeff_three_d, W_init, b_init, W1, b1, W2, b2, W_lin, b_lin):
    import ml_dtypes

    f32 = np.float32
    f16 = np.float16
    w1aug = np.concatenate([np.asarray(W1, f32), np.asarray(b1, f32)[None]], 0).astype(f16)
    w2aug = np.concatenate([np.asarray(W2, f32), np.asarray(b2, f32)[None]], 0).astype(f16)
    wlaug = np.concatenate([np.asarray(W_lin, f32), np.asarray(b_lin, f32)[None]], 0)
    wiaug = np.concatenate([np.asarray(W_init, f32), np.asarray(b_init, f32)[None]], 0)
    ident = np.eye(128, dtype=f32)
    ca = np.asarray(coeff_a, f32)
    cb = np.asarray(coeff_b, f32)
    cc = np.asarray(coeff_two_c, f32)
    cd = np.asarray(coeff_three_d, f32)
    maps = []
    for c in range(NCORES):
        sl = slice(c * BL, (c + 1) * BL)
        maps.append(
            {
                "a0": np.ascontiguousarray(ca[sl, 0, :]),
                "cb": np.ascontiguousarray(cb[sl].reshape(BL, -1)),
                "cc": np.ascontiguousarray(cc[sl].reshape(BL, -1)),
                "cd": np.ascontiguousarray(cd[sl].reshape(BL, -1)),
                "w1aug": w1aug,
                "w2aug": w2aug,
                "wlaug": wlaug,
                "wiaug": wiaug,
                "ident": ident,
                "identh": np.eye(128, dtype=f16),
                "ones": np.ones((1, 128), f32),
                "onesh": np.ones((1, 128), f16),
            }
        )
    return maps


def _install_ntff_hook():
    """Provide antenv.axon_hooks (missing in this image) so trace=True works."""
    import types, ctypes, contextlib

    try:
        from antenv.axon_hooks import get_axon_ntff_profile_hook  # noqa: F401
        return
    except ImportError:
        pass
    import antenv

    hook = None
    try:
        lib = ctypes.CDLL("/opt/axon/libaxon_pjrt.so")
        if hasattr(lib, "axon_start_nrt_profile"):
            lib.axon_start_nrt_profile.argtypes = [
                ctypes.POINTER(ctypes.c_int64),
                ctypes.c_size_t,
            ]
            lib.axon_start_nrt_profile.restype = ctypes.c_int64
            lib.axon_stop_nrt_profile.argtypes = [ctypes.c_char_p]
            lib.axon_stop_nrt_profile.restype = ctypes.c_int64

            @contextlib.contextmanager
            def _hook(output_dir, device_ids):
                import jax

                jax.devices()
                if device_ids:
                    ids = (ctypes.c_int64 * len(device_ids))(*device_ids)
                    rc = lib.axon_start_nrt_profile(ids, len(device_ids))
                else:
                    rc = lib.axon_start_nrt_profile(None, 0)
                if rc != 0:
                    raise RuntimeError(f"axon_start_nrt_profile rc={rc}")
                try:
                    yield
                finally:
                    n = lib.axon_stop_nrt_profile(str(output_dir).encode())
                    print(f"ntff profile: {n} file(s) written to {output_dir}")

            hook = _hook
    except OSError:
        pass
    mod = types.ModuleType("antenv.axon_hooks")
    mod.get_axon_ntff_profile_hook = lambda: hook
    mod.set_axon_ntff_profile_hook = lambda h: None
    sys.modules["antenv.axon_hooks"] = mod
    antenv.axon_hooks = mod


def kernel(times, coeff_a, coeff_b, coeff_two_c, coeff_three_d, final_index,
           W_init, b_init, W1, b1, W2, b2, W_lin, b_lin, trace=False):
    from concourse import bass_utils

    if trace:
        _install_ntff_hook()

    if "nc" not in _cache:
        _cache["nc"] = _build()
    nc = _cache["nc"]
    maps = _prep_maps(coeff_a, coeff_b, coeff_two_c, coeff_three_d,
                      W_init, b_init, W1, b1, W2, b2, W_lin, b_lin)
    res = bass_utils.run_bass_kernel_spmd(nc, maps, core_ids=list(range(NCORES)), trace=trace)
    _cache["last_res"] = res
    y = np.concatenate(
        [res.results[c]["y"].reshape(BL, T, OUT) for c in range(NCORES)], 0
    )
    return y.astype(np.float32)


# revision 9
# speedup vs baseline: 2.0299x; 1.1140x over previous
# Neural CDE (RK4 3/8-rule scan) Trainium2 Bass kernel.
# Self-contained: hardcodes shapes/sharding; callable as kernel(**inputs).
import sys

sys.path.insert(0, "/opt/trn_rl_repo")
import numpy as np

B, T, IN, HID, HH, OUT = 1024, 128, 16, 64, 64, 10
NCORES = 8
BL = B // NCORES  # batch per core
STEPS = T - 1

_cache = {}


def _get_mult_scan():
    """Custom DVE op: out[k] = cumsum(in0*in1) along free dim (fp32 state)."""
    import concourse.dve_ops as dve_ops
    from concourse.dve_ops import DveOp, get_dve_sub_opcode
    from concourse.dve_spec import Spec, Src0, Src1, scan, AluOp, lower
    from concourse.dve_uop import DveOpSpec

    name = "MULT_SCAN_NCDE"
    for o in dve_ops.OPS:
        if o.name == name:
            return o
    spec = Spec(
        body=scan(AluOp.ADD, Src0 * Src1),
        reference=lambda in0, in1, s0, s1, imm2: np.cumsum(
            (in0.astype(np.float32) * in1.astype(np.float32)), axis=-1
        ),
    )
    op = DveOp(name, spec, subdim=False, uops_sha={})
    dve_ops.OPS.append(op)
    dve_ops.CUSTOM_DVE_SPECS[name] = spec
    dve_ops._SUB_OPCODE_FOR_NAME[name] = (
        dve_ops._CUSTOM_DVE_ROW_BASE + len(dve_ops.OPS) - 1
    )
    for ver in ("v3", "v4"):
        try:
            ds = DveOpSpec(
                name=name,
                opcode=get_dve_sub_opcode(name),
                uops=lower(spec, ver=ver),
                rd1_en=True,
            )
            op.uops_sha[ver] = ds.sha(ver)
        except Exception as e:
            print("mult_scan sha", ver, "failed:", e)
    return op


def _build(steps=STEPS, use_f32r=True, half_dt="float16"):
    import concourse.bacc as bacc
    import concourse.mybir as mybir
    from concourse.tile import TileContext

    fp32 = mybir.dt.float32
    f32r = mybir.dt.float32r if use_f32r else fp32
    fp16 = getattr(mybir.dt, half_dt)
    AF = mybir.ActivationFunctionType
    OP = mybir.AluOpType
    AX = mybir.AxisListType

    mscan = _get_mult_scan()
    nc = bacc.Bacc("TRN2", target_bir_lowering=False, debug=False, num_devices=NCORES)

    # ---- DRAM I/O (per-core shards; weights replicated) ----
    d_a0 = nc.dram_tensor("a0", (BL, IN), fp32, kind="ExternalInput")
    d_cb = nc.dram_tensor("cb", (BL, STEPS * IN), fp32, kind="ExternalInput")
    d_cc = nc.dram_tensor("cc", (BL, STEPS * IN), fp32, kind="ExternalInput")
    d_cd = nc.dram_tensor("cd", (BL, STEPS * IN), fp32, kind="ExternalInput")
    d_w1 = nc.dram_tensor("w1aug", (HID + 1, HH), fp16, kind="ExternalInput")
    d_w2 = nc.dram_tensor("w2aug", (HH + 1, HID * IN), fp16, kind="ExternalInput")
    d_wl = nc.dram_tensor("wlaug", (HID + 1, OUT), fp32, kind="ExternalInput")
    d_wi = nc.dram_tensor("wiaug", (IN + 1, HID), fp32, kind="ExternalInput")
    d_id = nc.dram_tensor("ident", (128, 128), fp32, kind="ExternalInput")
    d_idh = nc.dram_tensor("identh", (128, 128), fp16, kind="ExternalInput")
    d_ones = nc.dram_tensor("ones", (1, 128), fp32, kind="ExternalInput")
    d_onesh = nc.dram_tensor("onesh", (1, 128), fp16, kind="ExternalInput")
    d_y = nc.dram_tensor("y", (BL, T * OUT), fp32, kind="ExternalOutput")

    NC2 = STEPS * IN  # 2032
    H2 = HID // 2  # 32 (column-split half of h)

    with TileContext(nc) as tc, nc.allow_low_precision("fp16 f-path; fp32 state"):
        with (
            tc.tile_pool(name="const", bufs=1) as constp,
            tc.tile_pool(name="state", bufs=1) as statep,
            tc.tile_pool(name="big", bufs=1) as bigp,
            tc.tile_pool(name="fpool", bufs=3) as fpool,
            tc.tile_pool(name="hpsum", bufs=2, space="PSUM") as hpsum,
            tc.tile_pool(name="fpsum", bufs=2, space="PSUM") as fpsum,
            tc.tile_pool(name="tpsum", bufs=2, space="PSUM") as tpsum,
            tc.tile_pool(name="ypsum", bufs=1, space="PSUM") as ypsum,
        ):
            # ---- constants ----
            w1 = constp.tile([HID + 1, HH], fp16, tag="w1")
            w2 = constp.tile([HH + 1, HID * IN], fp16, tag="w2")
            wl = constp.tile([HID + 1, OUT], fp32, tag="wl")
            wi = constp.tile([IN + 1, HID], fp32, tag="wi")
            ident = constp.tile([128, 128], fp32, tag="ident")
            identh = constp.tile([128, 128], fp16, tag="identh")
            nc.sync.dma_start(w1[:], d_w1.ap()[:])
            nc.sync.dma_start(w2[:], d_w2.ap()[:])
            nc.sync.dma_start(wl[:], d_wl.ap()[:])
            nc.sync.dma_start(wi[:], d_wi.ap()[:])
            nc.sync.dma_start(ident[:], d_id.ap()[:])
            nc.sync.dma_start(identh[:], d_idh.ap()[:])

            # ---- spline derivative precompute: u, v, w (fp16) ----
            cb = bigp.tile([BL, NC2], fp32, tag="cb")
            cc = bigp.tile([BL, NC2], fp32, tag="cc")
            cd = bigp.tile([BL, NC2], fp32, tag="cd")
            tt0 = bigp.tile([BL, NC2], fp32, tag="tt0")
            uu = bigp.tile([BL, NC2], fp16, tag="uu")
            vv = bigp.tile([BL, NC2], fp16, tag="vv")
            ww = bigp.tile([BL, NC2], fp16, tag="ww")
            db0 = bigp.tile([BL, IN], fp16, tag="db0")
            nc.sync.dma_start(cb[:], d_cb.ap()[:])
            nc.sync.dma_start(cc[:], d_cc.ap()[:])
            nc.sync.dma_start(cd[:], d_cd.ap()[:])
            stt = nc.vector.scalar_tensor_tensor
            # u = b + c + d ; v = b + c/3 + d/9 ; w = b + 2c/3 + 4d/9
            stt(tt0[:], cc[:], 1.0, cb[:], OP.mult, OP.add)
            stt(uu[:], cd[:], 1.0, tt0[:], OP.mult, OP.add)
            stt(tt0[:], cc[:], 1.0 / 3.0, cb[:], OP.mult, OP.add)
            stt(vv[:], cd[:], 1.0 / 9.0, tt0[:], OP.mult, OP.add)
            stt(tt0[:], cc[:], 2.0 / 3.0, cb[:], OP.mult, OP.add)
            stt(ww[:], cd[:], 4.0 / 9.0, tt0[:], OP.mult, OP.add)
            nc.vector.tensor_copy(db0[:], cb[:, 0:IN])

            # ---- state tiles ----
            zT = statep.tile([HID + 1, BL], fp32, tag="zT")  # master state (T-layout, aug)
            zmid = statep.tile([HID + 1, BL], fp16, tag="zmid")
            zTh = statep.tile([HID + 1, BL], fp16, tag="zTh")
            hT = statep.tile([HH + 1, BL], fp16, tag="hT")
            k1 = statep.tile([HID, BL], fp32, tag="k1")
            k2 = statep.tile([HID, BL], fp32, tag="k2")
            k3 = statep.tile([HID, BL], fp32, tag="k3")
            t1 = statep.tile([HID, BL], fp32, tag="t1")
            t2a = statep.tile([HID, BL], fp32, tag="t2a")
            t2b = statep.tile([HID, BL], fp32, tag="t2b")
            t3a = statep.tile([HID, BL], fp32, tag="t3a")
            t3b = statep.tile([HID, BL], fp32, tag="t3b")
            t3c = statep.tile([HID, BL], fp32, tag="t3c")
            sc0 = statep.tile([BL, 513], fp32, tag="sc0")
            sc1 = statep.tile([BL, 513], fp32, tag="sc1")
            nc.vector.memset(sc0[:, 0:1], 0.0)
            nc.vector.memset(sc1[:, 0:1], 0.0)
            y_sb = statep.tile([BL, T * OUT], fp32, tag="y_sb")
            nc.sync.dma_start(zT[HID : HID + 1, :], d_ones.ap()[:])
            nc.sync.dma_start(zmid[HID : HID + 1, :], d_onesh.ap()[:])
            nc.sync.dma_start(zTh[HID : HID + 1, :], d_onesh.ap()[:])
            nc.sync.dma_start(hT[HH : HH + 1, :], d_onesh.ap()[:])

            # ---- z0 = a0 @ W_init + b_init ----
            with tc.tile_pool(name="prol", bufs=1) as prol:
                a0 = prol.tile([BL, IN], fp32, tag="a0")
                a0T = prol.tile([IN + 1, BL], fp32, tag="a0T")
                nc.sync.dma_start(a0[:], d_a0.ap()[:])
                a0Tp = tpsum.tile([IN, BL], fp32, tag="ftp")
                nc.tensor.transpose(a0Tp[:], a0[:], ident[:])
                nc.vector.tensor_copy(a0T[0:IN, :], a0Tp[:])
                nc.sync.dma_start(a0T[IN : IN + 1, :], d_ones.ap()[:])
                z0p = tpsum.tile([HID, BL], fp32, tag="ftp")
                nc.tensor.matmul(z0p[:], wi[:], a0T[:], start=True, stop=True)
                nc.vector.tensor_copy(zT[0:HID, :], z0p[:])
                nc.vector.tensor_copy(zTh[0:HID, :], z0p[:])

            # ---- y for t=0 ----
            yp = ypsum.tile([BL, OUT], fp32, tag="yp")
            nc.tensor.matmul(yp[:], zT[:], wl[:], start=True, stop=True)
            nc.scalar.activation(y_sb[:, 0:OUT], yp[:], AF.Copy)

            kk = (k1, k2, k3, None)

            # ---- main RK4 scan ----
            for t in range(steps):
                for s in range(4):
                    src = zTh if s == 0 else zmid
                    if s == 1:
                        stt(t1[:], k1[:], -1.0 / 3.0, zT[0:HID, :], OP.mult, OP.add)
                    elif s == 2:
                        stt(t2a[:], k1[:], 1.0, zT[0:HID, :], OP.mult, OP.add)
                        stt(t2b[:], k2[:], -1.0, t2a[:], OP.mult, OP.add)
                    elif s == 3:
                        stt(t3a[:], k2[:], 1.0, k3[:], OP.mult, OP.add)
                        stt(t3b[:], t3a[:], 3.0, k1[:], OP.mult, OP.add)
                    # mm1 (fp32) + relu (-> f32r for the mm2 weights-side)
                    hp = hpsum.tile([HH, BL], fp32, tag="hp")
                    nc.tensor.matmul(hp[:], w1[:], src[:], start=True, stop=True)
                    nc.scalar.activation(hT[0:HH, :], hp[:], AF.Relu)
                    # dX slice for this stage
                    if s == 0:
                        dx = db0[:] if t == 0 else uu[:, (t - 1) * IN : t * IN]
                    elif s == 1:
                        dx = vv[:, t * IN : (t + 1) * IN]
                    elif s == 2:
                        dx = ww[:, t * IN : (t + 1) * IN]
                    else:
                        dx = uu[:, t * IN : (t + 1) * IN]
                    dx3 = dx.unsqueeze(1).broadcast_to([BL, H2, IN])
                    F = fpool.tile([BL, HID], fp16, tag="F")
                    ftp = tpsum.tile([HID, BL], fp16, tag="ftp")
                    # column-split halves pipeline mm2->tanh->mult->reduce->transpose
                    for ch in range(2):
                        cols = slice(ch * 512, (ch + 1) * 512)
                        fpp = fpsum.tile([BL, 512], fp32, tag="fp")
                        nc.tensor.matmul(fpp[:], hT[:], w2[:, cols], start=True, stop=True)
                        f_sb = fpool.tile([BL, 512], fp16, tag="f_sb")
                        nc.scalar.activation(f_sb[:], fpp[:], AF.Tanh)
                        sc = sc0 if ch == 0 else sc1
                        nc.vector._custom_dve(
                            mscan, out=sc[:, 1:513], in0=f_sb[:], in1=dx3
                        )
                        scg = sc[:, 1:513].rearrange("p (g i) -> p g i", i=IN)
                        sclo = sc[:, 0:512].rearrange("p (g i) -> p g i", i=IN)
                        Fh = F[:, ch * H2 : (ch + 1) * H2]
                        nc.vector.tensor_tensor(
                            Fh.unsqueeze(2), scg[:, :, IN - 1 : IN], sclo[:, :, 0:1],
                            OP.subtract,
                        )
                        nc.tensor.transpose(
                            ftp[ch * H2 : (ch + 1) * H2, :], Fh, identh[:]
                        )
                    # combos (fp32 state updates)
                    if s < 3:
                        nc.scalar.activation(kk[s][:], ftp[:], AF.Copy)
                    if s == 0:
                        # za = z + k1/3 (k1 read straight from psum)
                        stt(zmid[0:HID, :], ftp[:], 1.0 / 3.0, zT[0:HID, :], OP.mult, OP.add)
                    elif s == 1:
                        # zb = z - k1/3 + k2 ; t1 precomputed off-chain
                        stt(zmid[0:HID, :], ftp[:], 1.0, t1[:], OP.mult, OP.add)
                    elif s == 2:
                        # zc = z + k1 - k2 + k3 ; t2b precomputed off-chain
                        stt(zmid[0:HID, :], ftp[:], 1.0, t2b[:], OP.mult, OP.add)
                    else:
                        # zn = z + (k1 + 3(k2+k3) + k4)/8 ; t3b precomputed
                        stt(t3c[:], ftp[:], 1.0, t3b[:], OP.mult, OP.add)
                        stt(zTh[0:HID, :], t3c[:], 0.125, zT[0:HID, :], OP.mult, OP.add)
                        stt(zT[0:HID, :], t3c[:], 0.125, zT[0:HID, :], OP.mult, OP.add)
                # y_{t+1} = z @ W_lin + b_lin
                yp = ypsum.tile([BL, OUT], fp32, tag="yp")
                nc.tensor.matmul(yp[:], zT[:], wl[:], start=True, stop=True)
                nc.scalar.activation(y_sb[:, (t + 1) * OUT : (t + 2) * OUT], yp[:], AF.Copy)

            nc.sync.dma_start(d_y.ap()[:], y_sb[:])

    nc.compile()
    return nc


def _prep_maps(coeff_a, coeff_b, coeff_two_c, coeff_three_d, W_init, b_init, W1, b1, W2, b2, W_lin, b_lin):
    import ml_dtypes

    f32 = np.float32
    f16 = np.float16
    w1aug = np.concatenate([np.asarray(W1, f32), np.asarray(b1, f32)[None]], 0).astype(f16)
    w2aug = np.concatenate([np.asarray(W2, f32), np.asarray(b2, f32)[None]], 0).astype(f16)
    wlaug = np.concatenate([np.asarray(W_lin, f32), np.asarray(b_lin, f32)[None]], 0)
    wiaug = np.concatenate([np.asarray(W_init, f32), np.asarray(b_init, f32)[None]], 0)
    ident = np.eye(128, dtype=f32)
    ca = np.asarray(coeff_a, f32)
    cb = np.asarray(coeff_b, f32)
    cc = np.asarray(coeff_two_c, f32)
    cd = np.asarray(coeff_three_d, f32)
    maps = []
    for c in range(NCORES):
        sl = slice(c * BL, (c + 1) * BL)
        maps.append(
            {
                "a0": np.ascontiguousarray(ca[sl, 0, :]),
                "cb": np.ascontiguousarray(cb[sl].reshape(BL, -1)),
                "cc": np.ascontiguousarray(cc[sl].reshape(BL, -1)),
                "cd": np.ascontiguousarray(cd[sl].reshape(BL, -1)),
                "w1aug": w1aug,
                "w2aug": w2aug,
                "wlaug": wlaug,
                "wiaug": wiaug,
                "ident": ident,
                "identh": np.eye(128, dtype=f16),
                "ones": np.ones((1, 128), f32),
                "onesh": np.ones((1, 128), f16),
            }
        )
    return maps


def _install_ntff_hook():
    """Provide antenv.axon_hooks (missing in this image) so trace=True works."""
    import types, ctypes, contextlib

    try:
        from antenv.axon_hooks import get_axon_ntff_profile_hook  # noqa: F401
        return
    except ImportError:
        pass
    import antenv

    hook = None
    try:
        lib = ctypes.CDLL("/opt/axon/libaxon_pjrt.so")
        if hasattr(lib, "axon_start_nrt_profile"):
            lib.axon_start_nrt_profile.argtypes = [
                ctypes.POINTER(ctypes.c_int64),
                ctypes.c_size_t,
            ]
            lib.axon_start_nrt_profile.restype = ctypes.c_int64
            lib.axon_stop_nrt_profile.argtypes = [ctypes.c_char_p]
            lib.axon_stop_nrt_profile.restype = ctypes.c_int64

            @contextlib.contextmanager
            def _hook(output_dir, device_ids):
                import jax

                jax.devices()
                if device_ids:
                    ids = (ctypes.c_int64 * len(device_ids))(*device_ids)
                    rc = lib.axon_start_nrt_profile(ids, len(device_ids))
                else:
                    rc = lib.axon_start_nrt_profile(None, 0)
                if rc != 0:
                    raise RuntimeError(f"axon_start_nrt_profile rc={rc}")
                try:
                    yield
                finally:
                    n = lib.axon_stop_nrt_profile(str(output_dir).encode())
                    print(f"ntff profile: {n} file(s) written to {output_dir}")

            hook = _hook
    except OSError:
        pass
    mod = types.ModuleType("antenv.axon_hooks")
    mod.get_axon_ntff_profile_hook = lambda: hook
    mod.set_axon_ntff_profile_hook = lambda h: None
    sys.modules["antenv.axon_hooks"] = mod
    antenv.axon_hooks = mod


def kernel(times, coeff_a, coeff_b, coeff_two_c, coeff_three_d, final_index,
           W_init, b_init, W1, b1, W2, b2, W_lin, b_lin, trace=False):
    from concourse import bass_utils

    if trace:
        _install_ntff_hook()

    if "nc" not in _cache:
        _cache["nc"] = _build()
    nc = _cache["nc"]
    maps = _prep_maps(coeff_a, coeff_b, coeff_two_c, coeff_three_d,
                      W_init, b_init, W1, b1, W2, b2, W_lin, b_lin)
    res = bass_utils.run_bass_kernel_spmd(nc, maps, core_ids=list(range(NCORES)), trace=trace)
    _cache["last_res"] = res
    y = np.concatenate(
        [res.results[c]["y"].reshape(BL, T, OUT) for c in range(NCORES)], 0
    )
    return y.astype(np.float32)


# revision 10
# speedup vs baseline: 2.1592x; 1.0637x over previous
# Neural CDE (RK4 3/8-rule scan) Trainium2 Bass kernel.
# Self-contained: hardcodes shapes/sharding; callable as kernel(**inputs).
import sys

sys.path.insert(0, "/opt/trn_rl_repo")
import numpy as np

B, T, IN, HID, HH, OUT = 1024, 128, 16, 64, 64, 10
NCORES = 8
BL = B // NCORES  # batch per core
STEPS = T - 1

_cache = {}


def _get_mult_scan():
    """Custom DVE op: out[k] = cumsum(in0*in1) along free dim (fp32 state)."""
    import concourse.dve_ops as dve_ops
    from concourse.dve_ops import DveOp, get_dve_sub_opcode
    from concourse.dve_spec import Spec, Src0, Src1, scan, AluOp, lower
    from concourse.dve_uop import DveOpSpec

    name = "MULT_SCAN_NCDE"
    for o in dve_ops.OPS:
        if o.name == name:
            return o
    spec = Spec(
        body=scan(AluOp.ADD, Src0 * Src1),
        reference=lambda in0, in1, s0, s1, imm2: np.cumsum(
            (in0.astype(np.float32) * in1.astype(np.float32)), axis=-1
        ),
    )
    op = DveOp(name, spec, subdim=False, uops_sha={})
    dve_ops.OPS.append(op)
    dve_ops.CUSTOM_DVE_SPECS[name] = spec
    dve_ops._SUB_OPCODE_FOR_NAME[name] = (
        dve_ops._CUSTOM_DVE_ROW_BASE + len(dve_ops.OPS) - 1
    )
    for ver in ("v3", "v4"):
        try:
            ds = DveOpSpec(
                name=name,
                opcode=get_dve_sub_opcode(name),
                uops=lower(spec, ver=ver),
                rd1_en=True,
            )
            op.uops_sha[ver] = ds.sha(ver)
        except Exception as e:
            print("mult_scan sha", ver, "failed:", e)
    return op


def _build(steps=STEPS, use_f32r=True, half_dt="float16"):
    import concourse.bacc as bacc
    import concourse.mybir as mybir
    from concourse.tile import TileContext

    fp32 = mybir.dt.float32
    f32r = mybir.dt.float32r if use_f32r else fp32
    fp16 = getattr(mybir.dt, half_dt)
    AF = mybir.ActivationFunctionType
    OP = mybir.AluOpType
    AX = mybir.AxisListType

    mscan = _get_mult_scan()
    nc = bacc.Bacc("TRN2", target_bir_lowering=False, debug=False, num_devices=NCORES)

    # ---- DRAM I/O (per-core shards; weights replicated) ----
    d_a0 = nc.dram_tensor("a0", (BL, IN), fp32, kind="ExternalInput")
    d_cb = nc.dram_tensor("cb", (BL, STEPS * IN), fp32, kind="ExternalInput")
    d_cc = nc.dram_tensor("cc", (BL, STEPS * IN), fp32, kind="ExternalInput")
    d_cd = nc.dram_tensor("cd", (BL, STEPS * IN), fp32, kind="ExternalInput")
    d_w1 = nc.dram_tensor("w1aug", (HID + 1, HH), fp16, kind="ExternalInput")
    d_w2 = nc.dram_tensor("w2aug", (HH + 1, HID * IN), fp16, kind="ExternalInput")
    d_wl = nc.dram_tensor("wlaug", (HID + 1, OUT), fp32, kind="ExternalInput")
    d_wi = nc.dram_tensor("wiaug", (IN + 1, HID), fp32, kind="ExternalInput")
    d_id = nc.dram_tensor("ident", (128, 128), fp32, kind="ExternalInput")
    d_idh = nc.dram_tensor("identh", (128, 128), fp16, kind="ExternalInput")
    d_ones = nc.dram_tensor("ones", (1, 128), fp32, kind="ExternalInput")
    d_onesh = nc.dram_tensor("onesh", (1, 128), fp16, kind="ExternalInput")
    d_y = nc.dram_tensor("y", (BL, T * OUT), fp32, kind="ExternalOutput")

    NC2 = STEPS * IN  # 2032
    H2 = HID // 2  # 32 (column-split half of h)

    with TileContext(nc) as tc, nc.allow_low_precision("fp16 f-path; fp32 state"):
        with (
            tc.tile_pool(name="const", bufs=1) as constp,
            tc.tile_pool(name="state", bufs=1) as statep,
            tc.tile_pool(name="big", bufs=1) as bigp,
            tc.tile_pool(name="fpool", bufs=3) as fpool,
            tc.tile_pool(name="hpsum", bufs=2, space="PSUM") as hpsum,
            tc.tile_pool(name="fpsum", bufs=2, space="PSUM") as fpsum,
            tc.tile_pool(name="tpsum", bufs=2, space="PSUM") as tpsum,
            tc.tile_pool(name="ypsum", bufs=1, space="PSUM") as ypsum,
        ):
            # ---- constants ----
            w1 = constp.tile([HID + 1, HH], fp16, tag="w1")
            w2 = constp.tile([HH + 1, HID * IN], fp16, tag="w2")
            wl = constp.tile([HID + 1, OUT], fp32, tag="wl")
            wi = constp.tile([IN + 1, HID], fp32, tag="wi")
            ident = constp.tile([128, 128], fp32, tag="ident")
            identh = constp.tile([128, 128], fp16, tag="identh")
            nc.sync.dma_start(w1[:], d_w1.ap()[:])
            nc.sync.dma_start(w2[:], d_w2.ap()[:])
            nc.sync.dma_start(wl[:], d_wl.ap()[:])
            nc.sync.dma_start(wi[:], d_wi.ap()[:])
            nc.sync.dma_start(ident[:], d_id.ap()[:])
            nc.sync.dma_start(identh[:], d_idh.ap()[:])

            # ---- spline derivative precompute: u, v, w (fp16) ----
            cb = bigp.tile([BL, NC2], fp32, tag="cb")
            cc = bigp.tile([BL, NC2], fp32, tag="cc")
            cd = bigp.tile([BL, NC2], fp32, tag="cd")
            tt0 = bigp.tile([BL, NC2], fp32, tag="tt0")
            uu = bigp.tile([BL, NC2], fp16, tag="uu")
            vv = bigp.tile([BL, NC2], fp16, tag="vv")
            ww = bigp.tile([BL, NC2], fp16, tag="ww")
            db0 = bigp.tile([BL, IN], fp16, tag="db0")
            nc.sync.dma_start(cb[:], d_cb.ap()[:])
            nc.sync.dma_start(cc[:], d_cc.ap()[:])
            nc.sync.dma_start(cd[:], d_cd.ap()[:])
            stt = nc.vector.scalar_tensor_tensor
            # u = b + c + d ; v = b + c/3 + d/9 ; w = b + 2c/3 + 4d/9
            stt(tt0[:], cc[:], 1.0, cb[:], OP.mult, OP.add)
            stt(uu[:], cd[:], 1.0, tt0[:], OP.mult, OP.add)
            stt(tt0[:], cc[:], 1.0 / 3.0, cb[:], OP.mult, OP.add)
            stt(vv[:], cd[:], 1.0 / 9.0, tt0[:], OP.mult, OP.add)
            stt(tt0[:], cc[:], 2.0 / 3.0, cb[:], OP.mult, OP.add)
            stt(ww[:], cd[:], 4.0 / 9.0, tt0[:], OP.mult, OP.add)
            nc.vector.tensor_copy(db0[:], cb[:, 0:IN])

            # ---- state tiles ----
            zT = statep.tile([HID + 1, BL], fp32, tag="zT")  # master state (T-layout, aug)
            zmid = statep.tile([HID + 1, BL], fp16, tag="zmid")
            zTh = statep.tile([HID + 1, BL], fp16, tag="zTh")
            hT = statep.tile([HH + 1, BL], fp16, tag="hT")
            k1 = statep.tile([HID, BL], fp32, tag="k1")
            k2 = statep.tile([HID, BL], fp32, tag="k2")
            k3 = statep.tile([HID, BL], fp32, tag="k3")
            t1 = statep.tile([HID, BL], fp32, tag="t1")
            t2a = statep.tile([HID, BL], fp32, tag="t2a")
            t2b = statep.tile([HID, BL], fp32, tag="t2b")
            q0 = statep.tile([HID, BL], fp32, tag="q0")
            q1 = statep.tile([HID, BL], fp32, tag="q1")
            q2 = statep.tile([HID, BL], fp32, tag="q2")
            t3c = statep.tile([HID, BL], fp32, tag="t3c")
            zzero = statep.tile([HID, BL], fp32, tag="zzero")
            nc.vector.memset(zzero[:], 0.0)
            sc0 = statep.tile([BL, 513], fp32, tag="sc0")
            sc1 = statep.tile([BL, 513], fp32, tag="sc1")
            nc.vector.memset(sc0[:, 0:1], 0.0)
            nc.vector.memset(sc1[:, 0:1], 0.0)
            y_sb = statep.tile([BL, T * OUT], fp32, tag="y_sb")
            nc.sync.dma_start(zT[HID : HID + 1, :], d_ones.ap()[:])
            nc.sync.dma_start(zmid[HID : HID + 1, :], d_onesh.ap()[:])
            nc.sync.dma_start(zTh[HID : HID + 1, :], d_onesh.ap()[:])
            nc.sync.dma_start(hT[HH : HH + 1, :], d_onesh.ap()[:])

            # ---- z0 = a0 @ W_init + b_init ----
            with tc.tile_pool(name="prol", bufs=1) as prol:
                a0 = prol.tile([BL, IN], fp32, tag="a0")
                a0T = prol.tile([IN + 1, BL], fp32, tag="a0T")
                nc.sync.dma_start(a0[:], d_a0.ap()[:])
                a0Tp = tpsum.tile([IN, BL], fp32, tag="ftp")
                nc.tensor.transpose(a0Tp[:], a0[:], ident[:])
                nc.vector.tensor_copy(a0T[0:IN, :], a0Tp[:])
                nc.sync.dma_start(a0T[IN : IN + 1, :], d_ones.ap()[:])
                z0p = tpsum.tile([HID, BL], fp32, tag="ftp")
                nc.tensor.matmul(z0p[:], wi[:], a0T[:], start=True, stop=True)
                nc.vector.tensor_copy(zT[0:HID, :], z0p[:])
                nc.vector.tensor_copy(zTh[0:HID, :], z0p[:])

            # ---- y for t=0 ----
            yp = ypsum.tile([BL, OUT], fp32, tag="yp")
            nc.tensor.matmul(yp[:], zT[:], wl[:], start=True, stop=True)
            nc.scalar.activation(y_sb[:, 0:OUT], yp[:], AF.Copy)

            # ---- main RK4 scan ----
            for t in range(steps):
                for s in range(4):
                    src = zTh if s == 0 else zmid
                    # mm1 (fp32) + relu (-> f32r for the mm2 weights-side)
                    hp = hpsum.tile([HH, BL], fp32, tag="hp")
                    nc.tensor.matmul(hp[:], w1[:], src[:], start=True, stop=True)
                    nc.scalar.activation(hT[0:HH, :], hp[:], AF.Relu)
                    # dX slice for this stage
                    if s == 0:
                        dx = db0[:] if t == 0 else uu[:, (t - 1) * IN : t * IN]
                    elif s == 1:
                        dx = vv[:, t * IN : (t + 1) * IN]
                    elif s == 2:
                        dx = ww[:, t * IN : (t + 1) * IN]
                    else:
                        dx = uu[:, t * IN : (t + 1) * IN]
                    dx3 = dx.unsqueeze(1).broadcast_to([BL, H2, IN])
                    F = fpool.tile([BL, HID], fp16, tag="F")
                    ftp = tpsum.tile([HID, BL], fp16, tag="ftp")
                    # column-split halves pipeline mm2->tanh->mult->reduce->transpose
                    for ch in range(2):
                        cols = slice(ch * 512, (ch + 1) * 512)
                        fpp = fpsum.tile([BL, 512], fp32, tag="fp")
                        nc.tensor.matmul(fpp[:], hT[:], w2[:, cols], start=True, stop=True)
                        f_sb = fpool.tile([BL, 512], fp16, tag="f_sb")
                        nc.scalar.activation(f_sb[:], fpp[:], AF.Tanh)
                        sc = sc0 if ch == 0 else sc1
                        nc.vector._custom_dve(
                            mscan, out=sc[:, 1:513], in0=f_sb[:], in1=dx3
                        )
                        scg = sc[:, 1:513].rearrange("p (g i) -> p g i", i=IN)
                        sclo = sc[:, 0:512].rearrange("p (g i) -> p g i", i=IN)
                        Fh = F[:, ch * H2 : (ch + 1) * H2]
                        nc.vector.tensor_tensor(
                            Fh.unsqueeze(2), scg[:, :, IN - 1 : IN], sclo[:, :, 0:1],
                            OP.subtract,
                        )
                        nc.tensor.transpose(
                            ftp[ch * H2 : (ch + 1) * H2, :], Fh, identh[:]
                        )
                    # combos (fp32 state updates)
                    if s == 0:
                        # ON-chain: za = z + k1/3
                        stt(zmid[0:HID, :], ftp[:], 1.0 / 3.0, zT[0:HID, :], OP.mult, OP.add)
                        # off-chain prefixes from k1
                        stt(t1[:], ftp[:], -1.0 / 3.0, zT[0:HID, :], OP.mult, OP.add)
                        stt(t2a[:], ftp[:], 1.0, zT[0:HID, :], OP.mult, OP.add)
                        stt(q0[:], ftp[:], 0.125, zzero[:], OP.mult, OP.add)
                    elif s == 1:
                        # ON: zb = t1 + k2
                        stt(zmid[0:HID, :], ftp[:], 1.0, t1[:], OP.mult, OP.add)
                        stt(t2b[:], ftp[:], -1.0, t2a[:], OP.mult, OP.add)
                        stt(q1[:], ftp[:], 0.375, q0[:], OP.mult, OP.add)
                    elif s == 2:
                        # ON: zc = t2b + k3
                        stt(zmid[0:HID, :], ftp[:], 1.0, t2b[:], OP.mult, OP.add)
                        stt(q2[:], ftp[:], 0.375, q1[:], OP.mult, OP.add)
                    else:
                        # ON: zn = z + q2 + k4/8
                        stt(t3c[:], ftp[:], 0.125, q2[:], OP.mult, OP.add)
                        stt(zTh[0:HID, :], t3c[:], 1.0, zT[0:HID, :], OP.mult, OP.add)
                        stt(zT[0:HID, :], t3c[:], 1.0, zT[0:HID, :], OP.mult, OP.add)
                # y_{t+1} = z @ W_lin + b_lin
                yp = ypsum.tile([BL, OUT], fp32, tag="yp")
                nc.tensor.matmul(yp[:], zT[:], wl[:], start=True, stop=True)
                nc.scalar.activation(y_sb[:, (t + 1) * OUT : (t + 2) * OUT], yp[:], AF.Copy)

            nc.sync.dma_start(d_y.ap()[:], y_sb[:])

    nc.compile()
    return nc


def _prep_maps(coeff_a, coeff_b, coeff_two_c, coeff_three_d, W_init, b_init, W1, b1, W2, b2, W_lin, b_lin):
    import ml_dtypes

    f32 = np.float32
    f16 = np.float16
    w1aug = np.concatenate([np.asarray(W1, f32), np.asarray(b1, f32)[None]], 0).astype(f16)
    w2aug = np.concatenate([np.asarray(W2, f32), np.asarray(b2, f32)[None]], 0).astype(f16)
    wlaug = np.concatenate([np.asarray(W_lin, f32), np.asarray(b_lin, f32)[None]], 0)
    wiaug = np.concatenate([np.asarray(W_init, f32), np.asarray(b_init, f32)[None]], 0)
    ident = np.eye(128, dtype=f32)
    ca = np.asarray(coeff_a, f32)
    cb = np.asarray(coeff_b, f32)
    cc = np.asarray(coeff_two_c, f32)
    cd = np.asarray(coeff_three_d, f32)
    maps = []
    for c in range(NCORES):
        sl = slice(c * BL, (c + 1) * BL)
        maps.append(
            {
                "a0": np.ascontiguousarray(ca[sl, 0, :]),
                "cb": np.ascontiguousarray(cb[sl].reshape(BL, -1)),
                "cc": np.ascontiguousarray(cc[sl].reshape(BL, -1)),
                "cd": np.ascontiguousarray(cd[sl].reshape(BL, -1)),
                "w1aug": w1aug,
                "w2aug": w2aug,
                "wlaug": wlaug,
                "wiaug": wiaug,
                "ident": ident,
                "identh": np.eye(128, dtype=f16),
                "ones": np.ones((1, 128), f32),
                "onesh": np.ones((1, 128), f16),
            }
        )
    return maps


def _install_ntff_hook():
    """Provide antenv.axon_hooks (missing in this image) so trace=True works."""
    import types, ctypes, contextlib

    try:
        from antenv.axon_hooks import get_axon_ntff_profile_hook  # noqa: F401
        return
    except ImportError:
        pass
    import antenv

    hook = None
    try:
        lib = ctypes.CDLL("/opt/axon/libaxon_pjrt.so")
        if hasattr(lib, "axon_start_nrt_profile"):
            lib.axon_start_nrt_profile.argtypes = [
                ctypes.POINTER(ctypes.c_int64),
                ctypes.c_size_t,
            ]
            lib.axon_start_nrt_profile.restype = ctypes.c_int64
            lib.axon_stop_nrt_profile.argtypes = [ctypes.c_char_p]
            lib.axon_stop_nrt_profile.restype = ctypes.c_int64

            @contextlib.contextmanager
            def _hook(output_dir, device_ids):
                import jax

                jax.devices()
                if device_ids:
                    ids = (ctypes.c_int64 * len(device_ids))(*device_ids)
                    rc = lib.axon_start_nrt_profile(ids, len(device_ids))
                else:
                    rc = lib.axon_start_nrt_profile(None, 0)
                if rc != 0:
                    raise RuntimeError(f"axon_start_nrt_profile rc={rc}")
                try:
                    yield
                finally:
                    n = lib.axon_stop_nrt_profile(str(output_dir).encode())
                    print(f"ntff profile: {n} file(s) written to {output_dir}")

            hook = _hook
    except OSError:
        pass
    mod = types.ModuleType("antenv.axon_hooks")
    mod.get_axon_ntff_profile_hook = lambda: hook
    mod.set_axon_ntff_profile_hook = lambda h: None
    sys.modules["antenv.axon_hooks"] = mod
    antenv.axon_hooks = mod


def kernel(times, coeff_a, coeff_b, coeff_two_c, coeff_three_d, final_index,
           W_init, b_init, W1, b1, W2, b2, W_lin, b_lin, trace=False):
    from concourse import bass_utils

    if trace:
        _install_ntff_hook()

    if "nc" not in _cache:
        _cache["nc"] = _build()
    nc = _cache["nc"]
    maps = _prep_maps(coeff_a, coeff_b, coeff_two_c, coeff_three_d,
                      W_init, b_init, W1, b1, W2, b2, W_lin, b_lin)
    res = bass_utils.run_bass_kernel_spmd(nc, maps, core_ids=list(range(NCORES)), trace=trace)
    _cache["last_res"] = res
    y = np.concatenate(
        [res.results[c]["y"].reshape(BL, T, OUT) for c in range(NCORES)], 0
    )
    return y.astype(np.float32)


# revision 12
# speedup vs baseline: 2.2091x; 1.0231x over previous
# Neural CDE (RK4 3/8-rule scan) Trainium2 Bass kernel.
# Self-contained: hardcodes shapes/sharding; callable as kernel(**inputs).
import sys

sys.path.insert(0, "/opt/trn_rl_repo")
import numpy as np

B, T, IN, HID, HH, OUT = 1024, 128, 16, 64, 64, 10
NCORES = 8
BL = B // NCORES  # batch per core
STEPS = T - 1

_cache = {}


def _get_mult_scan():
    """Custom DVE op: out[k] = cumsum(in0*in1) along free dim (fp32 state)."""
    import concourse.dve_ops as dve_ops
    from concourse.dve_ops import DveOp, get_dve_sub_opcode
    from concourse.dve_spec import Spec, Src0, Src1, scan, AluOp, lower
    from concourse.dve_uop import DveOpSpec

    name = "MULT_SCAN_NCDE"
    for o in dve_ops.OPS:
        if o.name == name:
            return o
    spec = Spec(
        body=scan(AluOp.ADD, Src0 * Src1),
        reference=lambda in0, in1, s0, s1, imm2: np.cumsum(
            (in0.astype(np.float32) * in1.astype(np.float32)), axis=-1
        ),
    )
    op = DveOp(name, spec, subdim=False, uops_sha={})
    dve_ops.OPS.append(op)
    dve_ops.CUSTOM_DVE_SPECS[name] = spec
    dve_ops._SUB_OPCODE_FOR_NAME[name] = (
        dve_ops._CUSTOM_DVE_ROW_BASE + len(dve_ops.OPS) - 1
    )
    for ver in ("v3", "v4"):
        try:
            ds = DveOpSpec(
                name=name,
                opcode=get_dve_sub_opcode(name),
                uops=lower(spec, ver=ver),
                rd1_en=True,
            )
            op.uops_sha[ver] = ds.sha(ver)
        except Exception as e:
            print("mult_scan sha", ver, "failed:", e)
    return op


def _build(steps=STEPS, use_f32r=True, half_dt="float16"):
    import concourse.bacc as bacc
    import concourse.mybir as mybir
    from concourse.tile import TileContext

    fp32 = mybir.dt.float32
    f32r = mybir.dt.float32r if use_f32r else fp32
    fp16 = getattr(mybir.dt, half_dt)
    AF = mybir.ActivationFunctionType
    OP = mybir.AluOpType
    AX = mybir.AxisListType

    mscan = _get_mult_scan()
    nc = bacc.Bacc("TRN2", target_bir_lowering=False, debug=False, num_devices=NCORES)

    # ---- DRAM I/O (per-core shards; weights replicated) ----
    d_a0 = nc.dram_tensor("a0", (BL, IN), fp32, kind="ExternalInput")
    d_cb = nc.dram_tensor("cb", (BL, STEPS * IN), fp32, kind="ExternalInput")
    d_cc = nc.dram_tensor("cc", (BL, STEPS * IN), fp32, kind="ExternalInput")
    d_cd = nc.dram_tensor("cd", (BL, STEPS * IN), fp32, kind="ExternalInput")
    d_w1 = nc.dram_tensor("w1aug", (HID + 1, HH), fp16, kind="ExternalInput")
    d_w2 = nc.dram_tensor("w2aug", (HH + 1, HID * IN), fp16, kind="ExternalInput")
    d_wl = nc.dram_tensor("wlaug", (HID + 1, OUT), fp16, kind="ExternalInput")
    d_wi = nc.dram_tensor("wiaug", (IN + 1, HID), fp32, kind="ExternalInput")
    d_id = nc.dram_tensor("ident", (128, 128), fp32, kind="ExternalInput")
    d_idh = nc.dram_tensor("identh", (128, 128), fp16, kind="ExternalInput")
    d_ones = nc.dram_tensor("ones", (1, 128), fp32, kind="ExternalInput")
    d_onesh = nc.dram_tensor("onesh", (1, 128), fp16, kind="ExternalInput")
    d_y = nc.dram_tensor("y", (BL, T * OUT), fp32, kind="ExternalOutput")

    NC2 = STEPS * IN  # 2032
    H2 = HID // 2  # 32 (column-split half of h)

    with TileContext(nc) as tc, nc.allow_low_precision("fp16 f-path; fp32 state"):
        with (
            tc.tile_pool(name="const", bufs=1) as constp,
            tc.tile_pool(name="state", bufs=1) as statep,
            tc.tile_pool(name="big", bufs=1) as bigp,
            tc.tile_pool(name="fpool", bufs=3) as fpool,
            tc.tile_pool(name="hpsum", bufs=2, space="PSUM") as hpsum,
            tc.tile_pool(name="fpsum", bufs=2, space="PSUM") as fpsum,
            tc.tile_pool(name="tpsum", bufs=2, space="PSUM") as tpsum,
            tc.tile_pool(name="ypsum", bufs=1, space="PSUM") as ypsum,
        ):
            # ---- constants ----
            w1 = constp.tile([HID + 1, HH], fp16, tag="w1")
            w2 = constp.tile([HH + 1, HID * IN], fp16, tag="w2")
            wl = constp.tile([HID + 1, OUT], fp16, tag="wl")
            wi = constp.tile([IN + 1, HID], fp32, tag="wi")
            ident = constp.tile([128, 128], fp32, tag="ident")
            identh = constp.tile([128, 128], fp16, tag="identh")
            nc.sync.dma_start(w1[:], d_w1.ap()[:])
            nc.sync.dma_start(w2[:], d_w2.ap()[:])
            nc.sync.dma_start(wl[:], d_wl.ap()[:])
            nc.sync.dma_start(wi[:], d_wi.ap()[:])
            nc.sync.dma_start(ident[:], d_id.ap()[:])
            nc.sync.dma_start(identh[:], d_idh.ap()[:])

            # ---- spline derivative precompute: u, v, w (fp16) ----
            cb = bigp.tile([BL, NC2], fp32, tag="cb")
            cc = bigp.tile([BL, NC2], fp32, tag="cc")
            cd = bigp.tile([BL, NC2], fp32, tag="cd")
            tt0 = bigp.tile([BL, NC2], fp32, tag="tt0")
            uu = bigp.tile([BL, NC2], fp16, tag="uu")
            vv = bigp.tile([BL, NC2], fp16, tag="vv")
            ww = bigp.tile([BL, NC2], fp16, tag="ww")
            db0 = bigp.tile([BL, IN], fp16, tag="db0")
            nc.sync.dma_start(cb[:], d_cb.ap()[:])
            nc.sync.dma_start(cc[:], d_cc.ap()[:])
            nc.sync.dma_start(cd[:], d_cd.ap()[:])
            stt = nc.vector.scalar_tensor_tensor
            # u = b + c + d ; v = b + c/3 + d/9 ; w = b + 2c/3 + 4d/9
            stt(tt0[:], cc[:], 1.0, cb[:], OP.mult, OP.add)
            stt(uu[:], cd[:], 1.0, tt0[:], OP.mult, OP.add)
            stt(tt0[:], cc[:], 1.0 / 3.0, cb[:], OP.mult, OP.add)
            stt(vv[:], cd[:], 1.0 / 9.0, tt0[:], OP.mult, OP.add)
            stt(tt0[:], cc[:], 2.0 / 3.0, cb[:], OP.mult, OP.add)
            stt(ww[:], cd[:], 4.0 / 9.0, tt0[:], OP.mult, OP.add)
            nc.vector.tensor_copy(db0[:], cb[:, 0:IN])

            # ---- state tiles ----
            zT = statep.tile([HID + 1, BL], fp32, tag="zT")  # master state (T-layout, aug)
            zmid = statep.tile([HID + 1, BL], fp16, tag="zmid")
            zTh = statep.tile([HID + 1, BL], fp16, tag="zTh")
            hT = statep.tile([HH + 1, BL], fp16, tag="hT")
            k1 = statep.tile([HID, BL], fp32, tag="k1")
            k2 = statep.tile([HID, BL], fp32, tag="k2")
            k3 = statep.tile([HID, BL], fp32, tag="k3")
            t1 = statep.tile([HID, BL], fp32, tag="t1")
            t2a = statep.tile([HID, BL], fp32, tag="t2a")
            t2b = statep.tile([HID, BL], fp32, tag="t2b")
            q0 = statep.tile([HID, BL], fp32, tag="q0")
            q1 = statep.tile([HID, BL], fp32, tag="q1")
            q2 = statep.tile([HID, BL], fp32, tag="q2")
            t3c = statep.tile([HID, BL], fp32, tag="t3c")
            zzero = statep.tile([HID, BL], fp32, tag="zzero")
            nc.vector.memset(zzero[:], 0.0)
            ystage = statep.tile([BL, 51 * OUT], fp32, tag="ystage")
            sc0 = statep.tile([BL, 513], fp32, tag="sc0")
            sc1 = statep.tile([BL, 513], fp32, tag="sc1")
            nc.vector.memset(sc0[:, 0:1], 0.0)
            nc.vector.memset(sc1[:, 0:1], 0.0)
            nc.sync.dma_start(zT[HID : HID + 1, :], d_ones.ap()[:])
            nc.sync.dma_start(zmid[HID : HID + 1, :], d_onesh.ap()[:])
            nc.sync.dma_start(zTh[HID : HID + 1, :], d_onesh.ap()[:])
            nc.sync.dma_start(hT[HH : HH + 1, :], d_onesh.ap()[:])

            # ---- z0 = a0 @ W_init + b_init ----
            with tc.tile_pool(name="prol", bufs=1) as prol:
                a0 = prol.tile([BL, IN], fp32, tag="a0")
                a0T = prol.tile([IN + 1, BL], fp32, tag="a0T")
                nc.sync.dma_start(a0[:], d_a0.ap()[:])
                a0Tp = tpsum.tile([IN, BL], fp32, tag="ftp")
                nc.tensor.transpose(a0Tp[:], a0[:], ident[:])
                nc.vector.tensor_copy(a0T[0:IN, :], a0Tp[:])
                nc.sync.dma_start(a0T[IN : IN + 1, :], d_ones.ap()[:])
                z0p = tpsum.tile([HID, BL], fp32, tag="ftp")
                nc.tensor.matmul(z0p[:], wi[:], a0T[:], start=True, stop=True)
                nc.vector.tensor_copy(zT[0:HID, :], z0p[:])
                nc.vector.tensor_copy(zTh[0:HID, :], z0p[:])

            # ---- main RK4 scan ----
            CH = 51  # y steps accumulated per PSUM bank before DMA flush
            yp = None
            for t in range(steps):
                if t % CH == 0:
                    yp = ypsum.tile([BL, CH * OUT], fp32, tag="yp")
                slot = t % CH
                nc.tensor.matmul(
                    yp[:, slot * OUT : (slot + 1) * OUT], zTh[:], wl[:],
                    start=True, stop=True,
                )
                if slot == CH - 1:
                    c0 = (t // CH) * CH * OUT
                    nc.scalar.activation(ystage[:], yp[:], AF.Copy)
                    nc.sync.dma_start(d_y.ap()[:, c0 : c0 + CH * OUT], ystage[:])
                for s in range(4):
                    src = zTh if s == 0 else zmid
                    # mm1 (fp32) + relu (-> f32r for the mm2 weights-side)
                    hp = hpsum.tile([HH, BL], fp32, tag="hp")
                    nc.tensor.matmul(hp[:], w1[:], src[:], start=True, stop=True)
                    nc.scalar.activation(hT[0:HH, :], hp[:], AF.Relu)
                    # dX slice for this stage
                    if s == 0:
                        dx = db0[:] if t == 0 else uu[:, (t - 1) * IN : t * IN]
                    elif s == 1:
                        dx = vv[:, t * IN : (t + 1) * IN]
                    elif s == 2:
                        dx = ww[:, t * IN : (t + 1) * IN]
                    else:
                        dx = uu[:, t * IN : (t + 1) * IN]
                    dx3 = dx.unsqueeze(1).broadcast_to([BL, H2, IN])
                    F = fpool.tile([BL, HID], fp16, tag="F")
                    ftp = tpsum.tile([HID, BL], fp16, tag="ftp")
                    # column-split halves pipeline mm2->tanh->mult->reduce->transpose
                    for ch in range(2):
                        cols = slice(ch * 512, (ch + 1) * 512)
                        fpp = fpsum.tile([BL, 512], fp32, tag="fp")
                        nc.tensor.matmul(fpp[:], hT[:], w2[:, cols], start=True, stop=True)
                        f_sb = fpool.tile([BL, 512], fp16, tag="f_sb")
                        nc.scalar.activation(f_sb[:], fpp[:], AF.Tanh)
                        sc = sc0 if ch == 0 else sc1
                        nc.vector._custom_dve(
                            mscan, out=sc[:, 1:513], in0=f_sb[:], in1=dx3
                        )
                        scg = sc[:, 1:513].rearrange("p (g i) -> p g i", i=IN)
                        sclo = sc[:, 0:512].rearrange("p (g i) -> p g i", i=IN)
                        Fh = F[:, ch * H2 : (ch + 1) * H2]
                        nc.vector.tensor_tensor(
                            Fh.unsqueeze(2), scg[:, :, IN - 1 : IN], sclo[:, :, 0:1],
                            OP.subtract,
                        )
                    nc.tensor.transpose(ftp[:], F[:], identh[:])
                    # combos (fp32 state updates)
                    if s == 0:
                        # ON-chain: za = z + k1/3
                        stt(zmid[0:HID, :], ftp[:], 1.0 / 3.0, zT[0:HID, :], OP.mult, OP.add)
                        # off-chain prefixes from k1
                        stt(t1[:], ftp[:], -1.0 / 3.0, zT[0:HID, :], OP.mult, OP.add)
                        stt(t2a[:], ftp[:], 1.0, zT[0:HID, :], OP.mult, OP.add)
                        stt(q0[:], ftp[:], 0.125, zzero[:], OP.mult, OP.add)
                    elif s == 1:
                        # ON: zb = t1 + k2
                        stt(zmid[0:HID, :], ftp[:], 1.0, t1[:], OP.mult, OP.add)
                        stt(t2b[:], ftp[:], -1.0, t2a[:], OP.mult, OP.add)
                        stt(q1[:], ftp[:], 0.375, q0[:], OP.mult, OP.add)
                    elif s == 2:
                        # ON: zc = t2b + k3
                        stt(zmid[0:HID, :], ftp[:], 1.0, t2b[:], OP.mult, OP.add)
                        stt(q2[:], ftp[:], 0.375, q1[:], OP.mult, OP.add)
                        stt(t3c[:], q2[:], 1.0, zT[0:HID, :], OP.mult, OP.add)
                    else:
                        # ON: zn = (z + q2) + k4/8, one fused op
                        stt(zTh[0:HID, :], ftp[:], 0.125, t3c[:], OP.mult, OP.add)
                        stt(zT[0:HID, :], ftp[:], 0.125, t3c[:], OP.mult, OP.add)
            # final y entries: t = steps (last state) plus tail of last chunk
            tg = steps
            if tg % CH == 0:
                yp = ypsum.tile([BL, CH * OUT], fp32, tag="yp")
            slot = tg % CH
            nc.tensor.matmul(
                yp[:, slot * OUT : (slot + 1) * OUT], zTh[:], wl[:],
                start=True, stop=True,
            )
            c0 = (tg // CH) * CH * OUT
            nc.scalar.activation(
                ystage[:, 0 : (slot + 1) * OUT], yp[:, 0 : (slot + 1) * OUT], AF.Copy
            )
            nc.sync.dma_start(
                d_y.ap()[:, c0 : c0 + (slot + 1) * OUT], ystage[:, 0 : (slot + 1) * OUT]
            )

    nc.compile()
    return nc


def _prep_maps(coeff_a, coeff_b, coeff_two_c, coeff_three_d, W_init, b_init, W1, b1, W2, b2, W_lin, b_lin):
    import ml_dtypes

    f32 = np.float32
    f16 = np.float16
    w1aug = np.concatenate([np.asarray(W1, f32), np.asarray(b1, f32)[None]], 0).astype(f16)
    w2aug = np.concatenate([np.asarray(W2, f32), np.asarray(b2, f32)[None]], 0).astype(f16)
    wlaug = np.concatenate([np.asarray(W_lin, f32), np.asarray(b_lin, f32)[None]], 0).astype(f16)
    wiaug = np.concatenate([np.asarray(W_init, f32), np.asarray(b_init, f32)[None]], 0)
    ident = np.eye(128, dtype=f32)
    ca = np.asarray(coeff_a, f32)
    cb = np.asarray(coeff_b, f32)
    cc = np.asarray(coeff_two_c, f32)
    cd = np.asarray(coeff_three_d, f32)
    maps = []
    for c in range(NCORES):
        sl = slice(c * BL, (c + 1) * BL)
        maps.append(
            {
                "a0": np.ascontiguousarray(ca[sl, 0, :]),
                "cb": np.ascontiguousarray(cb[sl].reshape(BL, -1)),
                "cc": np.ascontiguousarray(cc[sl].reshape(BL, -1)),
                "cd": np.ascontiguousarray(cd[sl].reshape(BL, -1)),
                "w1aug": w1aug,
                "w2aug": w2aug,
                "wlaug": wlaug,
                "wiaug": wiaug,
                "ident": ident,
                "identh": np.eye(128, dtype=f16),
                "ones": np.ones((1, 128), f32),
                "onesh": np.ones((1, 128), f16),
            }
        )
    return maps


def _install_ntff_hook():
    """Provide antenv.axon_hooks (missing in this image) so trace=True works."""
    import types, ctypes, contextlib

    try:
        from antenv.axon_hooks import get_axon_ntff_profile_hook  # noqa: F401
        return
    except ImportError:
        pass
    import antenv

    hook = None
    try:
        lib = ctypes.CDLL("/opt/axon/libaxon_pjrt.so")
        if hasattr(lib, "axon_start_nrt_profile"):
            lib.axon_start_nrt_profile.argtypes = [
                ctypes.POINTER(ctypes.c_int64),
                ctypes.c_size_t,
            ]
            lib.axon_start_nrt_profile.restype = ctypes.c_int64
            lib.axon_stop_nrt_profile.argtypes = [ctypes.c_char_p]
            lib.axon_stop_nrt_profile.restype = ctypes.c_int64

            @contextlib.contextmanager
            def _hook(output_dir, device_ids):
                import jax

                jax.devices()
                if device_ids:
                    ids = (ctypes.c_int64 * len(device_ids))(*device_ids)
                    rc = lib.axon_start_nrt_profile(ids, len(device_ids))
                else:
                    rc = lib.axon_start_nrt_profile(None, 0)
                if rc != 0:
                    raise RuntimeError(f"axon_start_nrt_profile rc={rc}")
                try:
                    yield
                finally:
                    n = lib.axon_stop_nrt_profile(str(output_dir).encode())
                    print(f"ntff profile: {n} file(s) written to {output_dir}")

            hook = _hook
    except OSError:
        pass
    mod = types.ModuleType("antenv.axon_hooks")
    mod.get_axon_ntff_profile_hook = lambda: hook
    mod.set_axon_ntff_profile_hook = lambda h: None
    sys.modules["antenv.axon_hooks"] = mod
    antenv.axon_hooks = mod


def kernel(times, coeff_a, coeff_b, coeff_two_c, coeff_three_d, final_index,
           W_init, b_init, W1, b1, W2, b2, W_lin, b_lin, trace=False):
    from concourse import bass_utils

    if trace:
        _install_ntff_hook()

    if "nc" not in _cache:
        _cache["nc"] = _build()
    nc = _cache["nc"]
    maps = _prep_maps(coeff_a, coeff_b, coeff_two_c, coeff_three_d,
                      W_init, b_init, W1, b1, W2, b2, W_lin, b_lin)
    res = bass_utils.run_bass_kernel_spmd(nc, maps, core_ids=list(range(NCORES)), trace=trace)
    _cache["last_res"] = res
    y = np.concatenate(
        [res.results[c]["y"].reshape(BL, T, OUT) for c in range(NCORES)], 0
    )
    return y.astype(np.float32)


# revision 13
# speedup vs baseline: 2.2276x; 1.0084x over previous
# Neural CDE (RK4 3/8-rule scan) Trainium2 Bass kernel.
# Self-contained: hardcodes shapes/sharding; callable as kernel(**inputs).
import sys

sys.path.insert(0, "/opt/trn_rl_repo")
import numpy as np

B, T, IN, HID, HH, OUT = 1024, 128, 16, 64, 64, 10
NCORES = 8
BL = B // NCORES  # batch per core
STEPS = T - 1

_cache = {}


def _get_mult_scan():
    """Custom DVE op: out[k] = cumsum(in0*in1) along free dim (fp32 state)."""
    import concourse.dve_ops as dve_ops
    from concourse.dve_ops import DveOp, get_dve_sub_opcode
    from concourse.dve_spec import Spec, Src0, Src1, scan, AluOp, lower
    from concourse.dve_uop import DveOpSpec

    name = "MULT_SCAN_NCDE"
    for o in dve_ops.OPS:
        if o.name == name:
            return o
    spec = Spec(
        body=scan(AluOp.ADD, Src0 * Src1),
        reference=lambda in0, in1, s0, s1, imm2: np.cumsum(
            (in0.astype(np.float32) * in1.astype(np.float32)), axis=-1
        ),
    )
    op = DveOp(name, spec, subdim=False, uops_sha={})
    dve_ops.OPS.append(op)
    dve_ops.CUSTOM_DVE_SPECS[name] = spec
    dve_ops._SUB_OPCODE_FOR_NAME[name] = (
        dve_ops._CUSTOM_DVE_ROW_BASE + len(dve_ops.OPS) - 1
    )
    for ver in ("v3", "v4"):
        try:
            ds = DveOpSpec(
                name=name,
                opcode=get_dve_sub_opcode(name),
                uops=lower(spec, ver=ver),
                rd1_en=True,
            )
            op.uops_sha[ver] = ds.sha(ver)
        except Exception as e:
            print("mult_scan sha", ver, "failed:", e)
    return op


def _build(steps=STEPS, use_f32r=True, half_dt="float16"):
    import concourse.bacc as bacc
    import concourse.mybir as mybir
    from concourse.tile import TileContext

    fp32 = mybir.dt.float32
    f32r = mybir.dt.float32r if use_f32r else fp32
    fp16 = getattr(mybir.dt, half_dt)
    AF = mybir.ActivationFunctionType
    OP = mybir.AluOpType
    AX = mybir.AxisListType

    mscan = _get_mult_scan()
    nc = bacc.Bacc("TRN2", target_bir_lowering=False, debug=False, num_devices=NCORES)

    # ---- DRAM I/O (per-core shards; weights replicated) ----
    d_a0 = nc.dram_tensor("a0", (BL, IN), fp32, kind="ExternalInput")
    d_cb = nc.dram_tensor("cb", (BL, STEPS * IN), fp32, kind="ExternalInput")
    d_cc = nc.dram_tensor("cc", (BL, STEPS * IN), fp32, kind="ExternalInput")
    d_cd = nc.dram_tensor("cd", (BL, STEPS * IN), fp32, kind="ExternalInput")
    d_w1 = nc.dram_tensor("w1aug", (HID + 1, HH), fp16, kind="ExternalInput")
    d_w2 = nc.dram_tensor("w2aug", (HH + 1, HID * IN), fp16, kind="ExternalInput")
    d_wl = nc.dram_tensor("wlaug", (HID + 1, OUT), fp16, kind="ExternalInput")
    d_wi = nc.dram_tensor("wiaug", (IN + 1, HID), fp32, kind="ExternalInput")
    d_id = nc.dram_tensor("ident", (128, 128), fp32, kind="ExternalInput")
    d_idh = nc.dram_tensor("identh", (128, 128), fp16, kind="ExternalInput")
    d_ones = nc.dram_tensor("ones", (1, 128), fp32, kind="ExternalInput")
    d_onesh = nc.dram_tensor("onesh", (1, 128), fp16, kind="ExternalInput")
    d_y = nc.dram_tensor("y", (BL, T * OUT), fp32, kind="ExternalOutput")

    NC2 = STEPS * IN  # 2032
    H2 = HID // 2  # 32 (column-split half of h)

    with TileContext(nc) as tc, nc.allow_low_precision("fp16 f-path; fp32 state"):
        with (
            tc.tile_pool(name="const", bufs=1) as constp,
            tc.tile_pool(name="state", bufs=1) as statep,
            tc.tile_pool(name="big", bufs=1) as bigp,
            tc.tile_pool(name="fpool", bufs=3) as fpool,
            tc.tile_pool(name="hpsum", bufs=1, space="PSUM") as hpsum,
            tc.tile_pool(name="fpsum", bufs=2, space="PSUM") as fpsum,
            tc.tile_pool(name="tpsum", bufs=3, space="PSUM") as tpsum,
            tc.tile_pool(name="ypsum", bufs=1, space="PSUM") as ypsum,
        ):
            # ---- constants ----
            w1 = constp.tile([HID + 1, HH], fp16, tag="w1")
            w2 = constp.tile([HH + 1, HID * IN], fp16, tag="w2")
            wl = constp.tile([HID + 1, OUT], fp16, tag="wl")
            wi = constp.tile([IN + 1, HID], fp32, tag="wi")
            ident = constp.tile([128, 128], fp32, tag="ident")
            identh = constp.tile([128, 128], fp16, tag="identh")
            nc.sync.dma_start(w1[:], d_w1.ap()[:])
            nc.sync.dma_start(w2[:], d_w2.ap()[:])
            nc.sync.dma_start(wl[:], d_wl.ap()[:])
            nc.sync.dma_start(wi[:], d_wi.ap()[:])
            nc.sync.dma_start(ident[:], d_id.ap()[:])
            nc.sync.dma_start(identh[:], d_idh.ap()[:])

            # ---- spline derivative precompute: u, v, w (fp16) ----
            cb = bigp.tile([BL, NC2], fp32, tag="cb")
            cc = bigp.tile([BL, NC2], fp32, tag="cc")
            cd = bigp.tile([BL, NC2], fp32, tag="cd")
            tt0 = bigp.tile([BL, NC2], fp32, tag="tt0")
            uu = bigp.tile([BL, NC2], fp16, tag="uu")
            vv = bigp.tile([BL, NC2], fp16, tag="vv")
            ww = bigp.tile([BL, NC2], fp16, tag="ww")
            db0 = bigp.tile([BL, IN], fp16, tag="db0")
            nc.sync.dma_start(cb[:], d_cb.ap()[:])
            nc.sync.dma_start(cc[:], d_cc.ap()[:])
            nc.sync.dma_start(cd[:], d_cd.ap()[:])
            stt = nc.vector.scalar_tensor_tensor
            # u = b + c + d ; v = b + c/3 + d/9 ; w = b + 2c/3 + 4d/9
            stt(tt0[:], cc[:], 1.0, cb[:], OP.mult, OP.add)
            stt(uu[:], cd[:], 1.0, tt0[:], OP.mult, OP.add)
            stt(tt0[:], cc[:], 1.0 / 3.0, cb[:], OP.mult, OP.add)
            stt(vv[:], cd[:], 1.0 / 9.0, tt0[:], OP.mult, OP.add)
            stt(tt0[:], cc[:], 2.0 / 3.0, cb[:], OP.mult, OP.add)
            stt(ww[:], cd[:], 4.0 / 9.0, tt0[:], OP.mult, OP.add)
            nc.vector.tensor_copy(db0[:], cb[:, 0:IN])

            # ---- state tiles ----
            zT = statep.tile([HID + 1, BL], fp32, tag="zT")  # master state (T-layout, aug)
            zmid = statep.tile([HID + 1, BL], fp16, tag="zmid")
            zTh = statep.tile([HID + 1, BL], fp16, tag="zTh")
            hT = statep.tile([HH + 1, BL], fp16, tag="hT")
            k1 = statep.tile([HID, BL], fp32, tag="k1")
            k2 = statep.tile([HID, BL], fp32, tag="k2")
            k3 = statep.tile([HID, BL], fp32, tag="k3")
            t1 = statep.tile([HID, BL], fp32, tag="t1")
            t2a = statep.tile([HID, BL], fp32, tag="t2a")
            t2b = statep.tile([HID, BL], fp32, tag="t2b")
            q0 = statep.tile([HID, BL], fp32, tag="q0")
            q1 = statep.tile([HID, BL], fp32, tag="q1")
            q2 = statep.tile([HID, BL], fp32, tag="q2")
            t3c = statep.tile([HID, BL], fp32, tag="t3c")
            zzero = statep.tile([HID, BL], fp32, tag="zzero")
            nc.vector.memset(zzero[:], 0.0)
            ystage = statep.tile([BL, 51 * OUT], fp32, tag="ystage")
            sc0 = statep.tile([BL, 513], fp32, tag="sc0")
            sc1 = statep.tile([BL, 513], fp32, tag="sc1")
            nc.vector.memset(sc0[:, 0:1], 0.0)
            nc.vector.memset(sc1[:, 0:1], 0.0)
            nc.sync.dma_start(zT[HID : HID + 1, :], d_ones.ap()[:])
            nc.sync.dma_start(zmid[HID : HID + 1, :], d_onesh.ap()[:])
            nc.sync.dma_start(zTh[HID : HID + 1, :], d_onesh.ap()[:])
            nc.sync.dma_start(hT[HH : HH + 1, :], d_onesh.ap()[:])

            # ---- z0 = a0 @ W_init + b_init ----
            with tc.tile_pool(name="prol", bufs=1) as prol:
                a0 = prol.tile([BL, IN], fp32, tag="a0")
                a0T = prol.tile([IN + 1, BL], fp32, tag="a0T")
                nc.sync.dma_start(a0[:], d_a0.ap()[:])
                a0Tp = tpsum.tile([IN, BL], fp32, tag="ftp")
                nc.tensor.transpose(a0Tp[:], a0[:], ident[:])
                nc.vector.tensor_copy(a0T[0:IN, :], a0Tp[:])
                nc.sync.dma_start(a0T[IN : IN + 1, :], d_ones.ap()[:])
                z0p = tpsum.tile([HID, BL], fp32, tag="ftp")
                nc.tensor.matmul(z0p[:], wi[:], a0T[:], start=True, stop=True)
                nc.vector.tensor_copy(zT[0:HID, :], z0p[:])
                nc.vector.tensor_copy(zTh[0:HID, :], z0p[:])

            # ---- main RK4 scan ----
            CH = 51  # y steps accumulated per PSUM bank before DMA flush
            yp = None
            for t in range(steps):
                if t % CH == 0:
                    yp = ypsum.tile([BL, CH * OUT], fp32, tag="yp")
                slot = t % CH
                for s in range(4):
                    src = zTh if s == 0 else zmid
                    # mm1 (fp32) + relu (-> f32r for the mm2 weights-side)
                    hp = hpsum.tile([HH, BL], fp32, tag="hp")
                    nc.tensor.matmul(hp[:], w1[:], src[:], start=True, stop=True)
                    if s == 0:
                        # y_t = z_t @ W_lin (+b): PE-idle window, off the chain
                        nc.tensor.matmul(
                            yp[:, slot * OUT : (slot + 1) * OUT], zTh[:], wl[:],
                            start=True, stop=True,
                        )
                    nc.scalar.activation(hT[0:HH, :], hp[:], AF.Relu)
                    # dX slice for this stage
                    if s == 0:
                        dx = db0[:] if t == 0 else uu[:, (t - 1) * IN : t * IN]
                    elif s == 1:
                        dx = vv[:, t * IN : (t + 1) * IN]
                    elif s == 2:
                        dx = ww[:, t * IN : (t + 1) * IN]
                    else:
                        dx = uu[:, t * IN : (t + 1) * IN]
                    dx3 = dx.unsqueeze(1).broadcast_to([BL, H2, IN])
                    F = fpool.tile([BL, HID], fp16, tag="F")
                    ftp = tpsum.tile([HID, BL], fp16, tag="ftp")
                    # column-split halves pipeline mm2->tanh->mult->reduce->transpose
                    for ch in range(2):
                        cols = slice(ch * 512, (ch + 1) * 512)
                        fpp = fpsum.tile([BL, 512], fp32, tag="fp")
                        nc.tensor.matmul(fpp[:], hT[:], w2[:, cols], start=True, stop=True)
                        f_sb = fpool.tile([BL, 512], fp16, tag="f_sb")
                        nc.scalar.activation(f_sb[:], fpp[:], AF.Tanh)
                        sc = sc0 if ch == 0 else sc1
                        nc.vector._custom_dve(
                            mscan, out=sc[:, 1:513], in0=f_sb[:], in1=dx3
                        )
                        scg = sc[:, 1:513].rearrange("p (g i) -> p g i", i=IN)
                        sclo = sc[:, 0:512].rearrange("p (g i) -> p g i", i=IN)
                        Fh = F[:, ch * H2 : (ch + 1) * H2]
                        nc.vector.tensor_tensor(
                            Fh.unsqueeze(2), scg[:, :, IN - 1 : IN], sclo[:, :, 0:1],
                            OP.subtract,
                        )
                    nc.tensor.transpose(ftp[:], F[:], identh[:])
                    # combos (fp32 state updates)
                    if s == 0:
                        # ON-chain: za = z + k1/3
                        stt(zmid[0:HID, :], ftp[:], 1.0 / 3.0, zT[0:HID, :], OP.mult, OP.add)
                        # off-chain prefixes from k1
                        stt(t1[:], ftp[:], -1.0 / 3.0, zT[0:HID, :], OP.mult, OP.add)
                        stt(t2a[:], ftp[:], 1.0, zT[0:HID, :], OP.mult, OP.add)
                        stt(q0[:], ftp[:], 0.125, zzero[:], OP.mult, OP.add)
                    elif s == 1:
                        # ON: zb = t1 + k2
                        stt(zmid[0:HID, :], ftp[:], 1.0, t1[:], OP.mult, OP.add)
                        stt(t2b[:], ftp[:], -1.0, t2a[:], OP.mult, OP.add)
                        stt(q1[:], ftp[:], 0.375, q0[:], OP.mult, OP.add)
                    elif s == 2:
                        # ON: zc = t2b + k3
                        stt(zmid[0:HID, :], ftp[:], 1.0, t2b[:], OP.mult, OP.add)
                        stt(q2[:], ftp[:], 0.375, q1[:], OP.mult, OP.add)
                        stt(t3c[:], q2[:], 1.0, zT[0:HID, :], OP.mult, OP.add)
                    else:
                        # ON: zn = (z + q2) + k4/8, one fused op
                        stt(zTh[0:HID, :], ftp[:], 0.125, t3c[:], OP.mult, OP.add)
                        stt(zT[0:HID, :], ftp[:], 0.125, t3c[:], OP.mult, OP.add)
                if slot == CH - 1:
                    c0 = (t // CH) * CH * OUT
                    nc.scalar.activation(ystage[:], yp[:], AF.Copy)
                    nc.sync.dma_start(d_y.ap()[:, c0 : c0 + CH * OUT], ystage[:])
            # final y entries: t = steps (last state) plus tail of last chunk
            tg = steps
            if tg % CH == 0:
                yp = ypsum.tile([BL, CH * OUT], fp32, tag="yp")
            slot = tg % CH
            nc.tensor.matmul(
                yp[:, slot * OUT : (slot + 1) * OUT], zTh[:], wl[:],
                start=True, stop=True,
            )
            c0 = (tg // CH) * CH * OUT
            nc.scalar.activation(
                ystage[:, 0 : (slot + 1) * OUT], yp[:, 0 : (slot + 1) * OUT], AF.Copy
            )
            nc.sync.dma_start(
                d_y.ap()[:, c0 : c0 + (slot + 1) * OUT], ystage[:, 0 : (slot + 1) * OUT]
            )

    nc.compile()
    return nc


def _prep_maps(coeff_a, coeff_b, coeff_two_c, coeff_three_d, W_init, b_init, W1, b1, W2, b2, W_lin, b_lin):
    import ml_dtypes

    f32 = np.float32
    f16 = np.float16
    w1aug = np.concatenate([np.asarray(W1, f32), np.asarray(b1, f32)[None]], 0).astype(f16)
    w2aug = np.concatenate([np.asarray(W2, f32), np.asarray(b2, f32)[None]], 0).astype(f16)
    wlaug = np.concatenate([np.asarray(W_lin, f32), np.asarray(b_lin, f32)[None]], 0).astype(f16)
    wiaug = np.concatenate([np.asarray(W_init, f32), np.asarray(b_init, f32)[None]], 0)
    ident = np.eye(128, dtype=f32)
    ca = np.asarray(coeff_a, f32)
    cb = np.asarray(coeff_b, f32)
    cc = np.asarray(coeff_two_c, f32)
    cd = np.asarray(coeff_three_d, f32)
    maps = []
    for c in range(NCORES):
        sl = slice(c * BL, (c + 1) * BL)
        maps.append(
            {
                "a0": np.ascontiguousarray(ca[sl, 0, :]),
                "cb": np.ascontiguousarray(cb[sl].reshape(BL, -1)),
                "cc": np.ascontiguousarray(cc[sl].reshape(BL, -1)),
                "cd": np.ascontiguousarray(cd[sl].reshape(BL, -1)),
                "w1aug": w1aug,
                "w2aug": w2aug,
                "wlaug": wlaug,
                "wiaug": wiaug,
                "ident": ident,
                "identh": np.eye(128, dtype=f16),
                "ones": np.ones((1, 128), f32),
                "onesh": np.ones((1, 128), f16),
            }
        )
    return maps


def _install_ntff_hook():
    """Provide antenv.axon_hooks (missing in this image) so trace=True works."""
    import types, ctypes, contextlib

    try:
        from antenv.axon_hooks import get_axon_ntff_profile_hook  # noqa: F401
        return
    except ImportError:
        pass
    import antenv

    hook = None
    try:
        lib = ctypes.CDLL("/opt/axon/libaxon_pjrt.so")
        if hasattr(lib, "axon_start_nrt_profile"):
            lib.axon_start_nrt_profile.argtypes = [
                ctypes.POINTER(ctypes.c_int64),
                ctypes.c_size_t,
            ]
            lib.axon_start_nrt_profile.restype = ctypes.c_int64
            lib.axon_stop_nrt_profile.argtypes = [ctypes.c_char_p]
            lib.axon_stop_nrt_profile.restype = ctypes.c_int64

            @contextlib.contextmanager
            def _hook(output_dir, device_ids):
                import jax

                jax.devices()
                if device_ids:
                    ids = (ctypes.c_int64 * len(device_ids))(*device_ids)
                    rc = lib.axon_start_nrt_profile(ids, len(device_ids))
                else:
                    rc = lib.axon_start_nrt_profile(None, 0)
                if rc != 0:
                    raise RuntimeError(f"axon_start_nrt_profile rc={rc}")
                try:
                    yield
                finally:
                    n = lib.axon_stop_nrt_profile(str(output_dir).encode())
                    print(f"ntff profile: {n} file(s) written to {output_dir}")

            hook = _hook
    except OSError:
        pass
    mod = types.ModuleType("antenv.axon_hooks")
    mod.get_axon_ntff_profile_hook = lambda: hook
    mod.set_axon_ntff_profile_hook = lambda h: None
    sys.modules["antenv.axon_hooks"] = mod
    antenv.axon_hooks = mod


def kernel(times, coeff_a, coeff_b, coeff_two_c, coeff_three_d, final_index,
           W_init, b_init, W1, b1, W2, b2, W_lin, b_lin, trace=False):
    from concourse import bass_utils

    if trace:
        _install_ntff_hook()

    if "nc" not in _cache:
        _cache["nc"] = _build()
    nc = _cache["nc"]
    maps = _prep_maps(coeff_a, coeff_b, coeff_two_c, coeff_three_d,
                      W_init, b_init, W1, b1, W2, b2, W_lin, b_lin)
    res = bass_utils.run_bass_kernel_spmd(nc, maps, core_ids=list(range(NCORES)), trace=trace)
    _cache["last_res"] = res
    y = np.concatenate(
        [res.results[c]["y"].reshape(BL, T, OUT) for c in range(NCORES)], 0
    )
    return y.astype(np.float32)


# revision 14
# speedup vs baseline: 2.2812x; 1.0241x over previous
# Neural CDE (RK4 3/8-rule scan) Trainium2 Bass kernel.
# Self-contained: hardcodes shapes/sharding; callable as kernel(**inputs).
import sys

sys.path.insert(0, "/opt/trn_rl_repo")
import numpy as np

B, T, IN, HID, HH, OUT = 1024, 128, 16, 64, 64, 10
NCORES = 8
BL = B // NCORES  # batch per core
STEPS = T - 1

_cache = {}


def _get_mult_scan():
    """Custom DVE op: out[k] = cumsum(in0*in1) along free dim (fp32 state)."""
    import concourse.dve_ops as dve_ops
    from concourse.dve_ops import DveOp, get_dve_sub_opcode
    from concourse.dve_spec import Spec, Src0, Src1, scan, AluOp, lower
    from concourse.dve_uop import DveOpSpec

    name = "MULT_SCAN_NCDE"
    for o in dve_ops.OPS:
        if o.name == name:
            return o
    spec = Spec(
        body=scan(AluOp.ADD, Src0 * Src1),
        reference=lambda in0, in1, s0, s1, imm2: np.cumsum(
            (in0.astype(np.float32) * in1.astype(np.float32)), axis=-1
        ),
    )
    op = DveOp(name, spec, subdim=False, uops_sha={})
    dve_ops.OPS.append(op)
    dve_ops.CUSTOM_DVE_SPECS[name] = spec
    dve_ops._SUB_OPCODE_FOR_NAME[name] = (
        dve_ops._CUSTOM_DVE_ROW_BASE + len(dve_ops.OPS) - 1
    )
    for ver in ("v3", "v4"):
        try:
            ds = DveOpSpec(
                name=name,
                opcode=get_dve_sub_opcode(name),
                uops=lower(spec, ver=ver),
                rd1_en=True,
            )
            op.uops_sha[ver] = ds.sha(ver)
        except Exception as e:
            print("mult_scan sha", ver, "failed:", e)
    return op


def _build(steps=STEPS, use_f32r=True, half_dt="float16"):
    import concourse.bacc as bacc
    import concourse.mybir as mybir
    from concourse.tile import TileContext

    fp32 = mybir.dt.float32
    f32r = mybir.dt.float32r if use_f32r else fp32
    fp16 = getattr(mybir.dt, half_dt)
    AF = mybir.ActivationFunctionType
    OP = mybir.AluOpType
    AX = mybir.AxisListType

    mscan = _get_mult_scan()
    nc = bacc.Bacc("TRN2", target_bir_lowering=False, debug=False, num_devices=NCORES)

    # ---- DRAM I/O (per-core shards; weights replicated) ----
    d_a0 = nc.dram_tensor("a0", (BL, IN), fp32, kind="ExternalInput")
    d_cb = nc.dram_tensor("cb", (BL, STEPS * IN), fp32, kind="ExternalInput")
    d_cc = nc.dram_tensor("cc", (BL, STEPS * IN), fp32, kind="ExternalInput")
    d_cd = nc.dram_tensor("cd", (BL, STEPS * IN), fp32, kind="ExternalInput")
    d_w1 = nc.dram_tensor("w1aug", (HID + 1, HH), fp16, kind="ExternalInput")
    d_w2 = nc.dram_tensor("w2aug", (HH + 1, HID * IN), fp16, kind="ExternalInput")
    d_wl = nc.dram_tensor("wlaug", (HID + 1, OUT), fp16, kind="ExternalInput")
    d_wi = nc.dram_tensor("wiaug", (IN + 1, HID), fp32, kind="ExternalInput")
    d_id = nc.dram_tensor("ident", (128, 128), fp32, kind="ExternalInput")
    d_idh = nc.dram_tensor("identh", (128, 128), fp16, kind="ExternalInput")
    d_ones = nc.dram_tensor("ones", (1, 128), fp32, kind="ExternalInput")
    d_onesh = nc.dram_tensor("onesh", (1, 128), fp16, kind="ExternalInput")
    d_y = nc.dram_tensor("y", (BL, T * OUT), fp32, kind="ExternalOutput")

    NC2 = STEPS * IN  # 2032
    H2 = HID // 2  # 32 (column-split half of h)

    with TileContext(nc) as tc, nc.allow_low_precision("fp16 f-path; fp32 state"):
        with (
            tc.tile_pool(name="const", bufs=1) as constp,
            tc.tile_pool(name="state", bufs=1) as statep,
            tc.tile_pool(name="big", bufs=1) as bigp,
            tc.tile_pool(name="fpool", bufs=3) as fpool,
            tc.tile_pool(name="hpsum", bufs=1, space="PSUM") as hpsum,
            tc.tile_pool(name="fpsum", bufs=2, space="PSUM") as fpsum,
            tc.tile_pool(name="tpsum", bufs=3, space="PSUM") as tpsum,
            tc.tile_pool(name="ypsum", bufs=1, space="PSUM") as ypsum,
        ):
            # ---- constants ----
            w1 = constp.tile([HID + 1, HH], fp16, tag="w1")
            w2 = constp.tile([HH + 1, HID * IN], fp16, tag="w2")
            wl = constp.tile([HID + 1, OUT], fp16, tag="wl")
            wi = constp.tile([IN + 1, HID], fp32, tag="wi")
            ident = constp.tile([128, 128], fp32, tag="ident")
            identh = constp.tile([128, 128], fp16, tag="identh")
            nc.sync.dma_start(w1[:], d_w1.ap()[:])
            nc.sync.dma_start(w2[:], d_w2.ap()[:])
            nc.sync.dma_start(wl[:], d_wl.ap()[:])
            nc.sync.dma_start(wi[:], d_wi.ap()[:])
            nc.sync.dma_start(ident[:], d_id.ap()[:])
            nc.sync.dma_start(identh[:], d_idh.ap()[:])

            # ---- spline derivative precompute: u, v, w (fp16) ----
            cb = bigp.tile([BL, NC2], fp32, tag="cb")
            cc = bigp.tile([BL, NC2], fp32, tag="cc")
            cd = bigp.tile([BL, NC2], fp32, tag="cd")
            tt0 = bigp.tile([BL, NC2], fp32, tag="tt0")
            uu = bigp.tile([BL, NC2], fp16, tag="uu")
            vv = bigp.tile([BL, NC2], fp16, tag="vv")
            ww = bigp.tile([BL, NC2], fp16, tag="ww")
            db0 = bigp.tile([BL, IN], fp16, tag="db0")
            nc.sync.dma_start(cb[:], d_cb.ap()[:])
            nc.sync.dma_start(cc[:], d_cc.ap()[:])
            nc.sync.dma_start(cd[:], d_cd.ap()[:])
            stt = nc.vector.scalar_tensor_tensor
            # u = b + c + d ; v = b + c/3 + d/9 ; w = b + 2c/3 + 4d/9
            stt(tt0[:], cc[:], 1.0, cb[:], OP.mult, OP.add)
            stt(uu[:], cd[:], 1.0, tt0[:], OP.mult, OP.add)
            stt(tt0[:], cc[:], 1.0 / 3.0, cb[:], OP.mult, OP.add)
            stt(vv[:], cd[:], 1.0 / 9.0, tt0[:], OP.mult, OP.add)
            stt(tt0[:], cc[:], 2.0 / 3.0, cb[:], OP.mult, OP.add)
            stt(ww[:], cd[:], 4.0 / 9.0, tt0[:], OP.mult, OP.add)
            nc.vector.tensor_copy(db0[:], cb[:, 0:IN])

            # ---- state tiles ----
            zT = statep.tile([HID + 1, BL], fp32, tag="zT")  # master state (T-layout, aug)
            zmid = statep.tile([HID + 1, BL], fp16, tag="zmid")
            zTh = statep.tile([HID + 1, BL], fp16, tag="zTh")
            hT = statep.tile([HH + 1, BL], fp16, tag="hT")
            k1 = statep.tile([HID, BL], fp32, tag="k1")
            k2 = statep.tile([HID, BL], fp32, tag="k2")
            k3 = statep.tile([HID, BL], fp32, tag="k3")
            t1 = statep.tile([HID, BL], fp32, tag="t1")
            t2a = statep.tile([HID, BL], fp32, tag="t2a")
            t2b = statep.tile([HID, BL], fp32, tag="t2b")
            q0 = statep.tile([HID, BL], fp32, tag="q0")
            q1 = statep.tile([HID, BL], fp32, tag="q1")
            q2 = statep.tile([HID, BL], fp32, tag="q2")
            t3c = statep.tile([HID, BL], fp32, tag="t3c")
            zzero = statep.tile([HID, BL], fp32, tag="zzero")
            nc.vector.memset(zzero[:], 0.0)
            ystage = statep.tile([BL, 51 * OUT], fp32, tag="ystage")
            sc0 = statep.tile([BL, 513], fp32, tag="sc0")
            sc1 = statep.tile([BL, 513], fp32, tag="sc1")
            nc.vector.memset(sc0[:, 0:1], 0.0)
            nc.vector.memset(sc1[:, 0:1], 0.0)
            nc.sync.dma_start(zT[HID : HID + 1, :], d_ones.ap()[:])
            nc.sync.dma_start(zmid[HID : HID + 1, :], d_onesh.ap()[:])
            nc.sync.dma_start(zTh[HID : HID + 1, :], d_onesh.ap()[:])
            nc.sync.dma_start(hT[HH : HH + 1, :], d_onesh.ap()[:])

            # ---- z0 = a0 @ W_init + b_init ----
            with tc.tile_pool(name="prol", bufs=1) as prol:
                a0 = prol.tile([BL, IN], fp32, tag="a0")
                a0T = prol.tile([IN + 1, BL], fp32, tag="a0T")
                nc.sync.dma_start(a0[:], d_a0.ap()[:])
                a0Tp = tpsum.tile([IN, BL], fp32, tag="ftp")
                nc.tensor.transpose(a0Tp[:], a0[:], ident[:])
                nc.vector.tensor_copy(a0T[0:IN, :], a0Tp[:])
                nc.sync.dma_start(a0T[IN : IN + 1, :], d_ones.ap()[:])
                z0p = tpsum.tile([HID, BL], fp32, tag="ftp")
                nc.tensor.matmul(z0p[:], wi[:], a0T[:], start=True, stop=True)
                nc.vector.tensor_copy(zT[0:HID, :], z0p[:])
                nc.vector.tensor_copy(zTh[0:HID, :], z0p[:])

            # ---- main RK4 scan ----
            CH = 51  # y steps accumulated per PSUM bank before DMA flush
            yp = None
            pending = []
            for t in range(steps):
                if t % CH == 0:
                    yp = ypsum.tile([BL, CH * OUT], fp32, tag="yp")
                slot = t % CH
                for s in range(4):
                    src = zTh if s == 0 else zmid
                    # mm1 (fp32) + relu (-> f32r for the mm2 weights-side)
                    hp = hpsum.tile([HH, BL], fp32, tag="hp")
                    nc.tensor.matmul(hp[:], w1[:], src[:], start=True, stop=True)
                    if s == 0:
                        # y_t = z_t @ W_lin (+b): PE-idle window, off the chain
                        nc.tensor.matmul(
                            yp[:, slot * OUT : (slot + 1) * OUT], zTh[:], wl[:],
                            start=True, stop=True,
                        )
                    nc.vector.tensor_scalar_max(hT[0:HH, :], hp[:], 0.0)
                    for fn in pending:
                        fn()
                    pending = []
                    # dX slice for this stage
                    if s == 0:
                        dx = db0[:] if t == 0 else uu[:, (t - 1) * IN : t * IN]
                    elif s == 1:
                        dx = vv[:, t * IN : (t + 1) * IN]
                    elif s == 2:
                        dx = ww[:, t * IN : (t + 1) * IN]
                    else:
                        dx = uu[:, t * IN : (t + 1) * IN]
                    dx3 = dx.unsqueeze(1).broadcast_to([BL, H2, IN])
                    F = fpool.tile([BL, HID], fp16, tag="F")
                    ftp = tpsum.tile([HID, BL], fp16, tag="ftp")
                    # column-split halves pipeline mm2->tanh->mult->reduce->transpose
                    for ch in range(2):
                        cols = slice(ch * 512, (ch + 1) * 512)
                        fpp = fpsum.tile([BL, 512], fp32, tag="fp")
                        nc.tensor.matmul(fpp[:], hT[:], w2[:, cols], start=True, stop=True)
                        f_sb = fpool.tile([BL, 512], fp16, tag="f_sb")
                        nc.scalar.activation(f_sb[:], fpp[:], AF.Tanh)
                        sc = sc0 if ch == 0 else sc1
                        nc.vector._custom_dve(
                            mscan, out=sc[:, 1:513], in0=f_sb[:], in1=dx3
                        )
                        scg = sc[:, 1:513].rearrange("p (g i) -> p g i", i=IN)
                        sclo = sc[:, 0:512].rearrange("p (g i) -> p g i", i=IN)
                        Fh = F[:, ch * H2 : (ch + 1) * H2]
                        nc.vector.tensor_tensor(
                            Fh.unsqueeze(2), scg[:, :, IN - 1 : IN], sclo[:, :, 0:1],
                            OP.subtract,
                        )
                    nc.tensor.transpose(ftp[:], F[:], identh[:])
                    # combos (fp32 state updates)
                    if s == 0:
                        # ON-chain: za = z + k1/3
                        stt(zmid[0:HID, :], ftp[:], 1.0 / 3.0, zT[0:HID, :], OP.mult, OP.add)
                        # off-chain prefixes from k1 (emitted after next relu)
                        pending.append(lambda ftp=ftp: (
                            stt(t1[:], ftp[:], -1.0 / 3.0, zT[0:HID, :], OP.mult, OP.add),
                            stt(t2a[:], ftp[:], 1.0, zT[0:HID, :], OP.mult, OP.add),
                            stt(q0[:], ftp[:], 0.125, zzero[:], OP.mult, OP.add),
                        ))
                    elif s == 1:
                        # ON: zb = t1 + k2
                        stt(zmid[0:HID, :], ftp[:], 1.0, t1[:], OP.mult, OP.add)
                        pending.append(lambda ftp=ftp: (
                            stt(t2b[:], ftp[:], -1.0, t2a[:], OP.mult, OP.add),
                            stt(q1[:], ftp[:], 0.375, q0[:], OP.mult, OP.add),
                        ))
                    elif s == 2:
                        # ON: zc = t2b + k3
                        stt(zmid[0:HID, :], ftp[:], 1.0, t2b[:], OP.mult, OP.add)
                        pending.append(lambda ftp=ftp: (
                            stt(q2[:], ftp[:], 0.375, q1[:], OP.mult, OP.add),
                            stt(t3c[:], q2[:], 1.0, zT[0:HID, :], OP.mult, OP.add),
                        ))
                    else:
                        # ON: zn = (z + q2) + k4/8, one fused op
                        stt(zTh[0:HID, :], ftp[:], 0.125, t3c[:], OP.mult, OP.add)
                        pending.append(lambda ftp=ftp: (
                            stt(zT[0:HID, :], ftp[:], 0.125, t3c[:], OP.mult, OP.add),
                        ))
                if slot == CH - 1:
                    c0 = (t // CH) * CH * OUT
                    nc.scalar.activation(ystage[:], yp[:], AF.Copy)
                    nc.sync.dma_start(d_y.ap()[:, c0 : c0 + CH * OUT], ystage[:])
            for fn in pending:
                fn()
            pending = []
            # final y entries: t = steps (last state) plus tail of last chunk
            tg = steps
            if tg % CH == 0:
                yp = ypsum.tile([BL, CH * OUT], fp32, tag="yp")
            slot = tg % CH
            nc.tensor.matmul(
                yp[:, slot * OUT : (slot + 1) * OUT], zTh[:], wl[:],
                start=True, stop=True,
            )
            c0 = (tg // CH) * CH * OUT
            nc.scalar.activation(
                ystage[:, 0 : (slot + 1) * OUT], yp[:, 0 : (slot + 1) * OUT], AF.Copy
            )
            nc.sync.dma_start(
                d_y.ap()[:, c0 : c0 + (slot + 1) * OUT], ystage[:, 0 : (slot + 1) * OUT]
            )

    nc.compile()
    return nc


def _prep_maps(coeff_a, coeff_b, coeff_two_c, coeff_three_d, W_init, b_init, W1, b1, W2, b2, W_lin, b_lin):
    import ml_dtypes

    f32 = np.float32
    f16 = np.float16
    w1aug = np.concatenate([np.asarray(W1, f32), np.asarray(b1, f32)[None]], 0).astype(f16)
    w2aug = np.concatenate([np.asarray(W2, f32), np.asarray(b2, f32)[None]], 0).astype(f16)
    wlaug = np.concatenate([np.asarray(W_lin, f32), np.asarray(b_lin, f32)[None]], 0).astype(f16)
    wiaug = np.concatenate([np.asarray(W_init, f32), np.asarray(b_init, f32)[None]], 0)
    ident = np.eye(128, dtype=f32)
    ca = np.asarray(coeff_a, f32)
    cb = np.asarray(coeff_b, f32)
    cc = np.asarray(coeff_two_c, f32)
    cd = np.asarray(coeff_three_d, f32)
    maps = []
    for c in range(NCORES):
        sl = slice(c * BL, (c + 1) * BL)
        maps.append(
            {
                "a0": np.ascontiguousarray(ca[sl, 0, :]),
                "cb": np.ascontiguousarray(cb[sl].reshape(BL, -1)),
                "cc": np.ascontiguousarray(cc[sl].reshape(BL, -1)),
                "cd": np.ascontiguousarray(cd[sl].reshape(BL, -1)),
                "w1aug": w1aug,
                "w2aug": w2aug,
                "wlaug": wlaug,
                "wiaug": wiaug,
                "ident": ident,
                "identh": np.eye(128, dtype=f16),
                "ones": np.ones((1, 128), f32),
                "onesh": np.ones((1, 128), f16),
            }
        )
    return maps


def _install_ntff_hook():
    """Provide antenv.axon_hooks (missing in this image) so trace=True works."""
    import types, ctypes, contextlib

    try:
        from antenv.axon_hooks import get_axon_ntff_profile_hook  # noqa: F401
        return
    except ImportError:
        pass
    import antenv

    hook = None
    try:
        lib = ctypes.CDLL("/opt/axon/libaxon_pjrt.so")
        if hasattr(lib, "axon_start_nrt_profile"):
            lib.axon_start_nrt_profile.argtypes = [
                ctypes.POINTER(ctypes.c_int64),
                ctypes.c_size_t,
            ]
            lib.axon_start_nrt_profile.restype = ctypes.c_int64
            lib.axon_stop_nrt_profile.argtypes = [ctypes.c_char_p]
            lib.axon_stop_nrt_profile.restype = ctypes.c_int64

            @contextlib.contextmanager
            def _hook(output_dir, device_ids):
                import jax

                jax.devices()
                if device_ids:
                    ids = (ctypes.c_int64 * len(device_ids))(*device_ids)
                    rc = lib.axon_start_nrt_profile(ids, len(device_ids))
                else:
                    rc = lib.axon_start_nrt_profile(None, 0)
                if rc != 0:
                    raise RuntimeError(f"axon_start_nrt_profile rc={rc}")
                try:
                    yield
                finally:
                    n = lib.axon_stop_nrt_profile(str(output_dir).encode())
                    print(f"ntff profile: {n} file(s) written to {output_dir}")

            hook = _hook
    except OSError:
        pass
    mod = types.ModuleType("antenv.axon_hooks")
    mod.get_axon_ntff_profile_hook = lambda: hook
    mod.set_axon_ntff_profile_hook = lambda h: None
    sys.modules["antenv.axon_hooks"] = mod
    antenv.axon_hooks = mod


def kernel(times, coeff_a, coeff_b, coeff_two_c, coeff_three_d, final_index,
           W_init, b_init, W1, b1, W2, b2, W_lin, b_lin, trace=False):
    from concourse import bass_utils

    if trace:
        _install_ntff_hook()

    if "nc" not in _cache:
        _cache["nc"] = _build()
    nc = _cache["nc"]
    maps = _prep_maps(coeff_a, coeff_b, coeff_two_c, coeff_three_d,
                      W_init, b_init, W1, b1, W2, b2, W_lin, b_lin)
    res = bass_utils.run_bass_kernel_spmd(nc, maps, core_ids=list(range(NCORES)), trace=trace)
    _cache["last_res"] = res
    y = np.concatenate(
        [res.results[c]["y"].reshape(BL, T, OUT) for c in range(NCORES)], 0
    )
    return y.astype(np.float32)


# revision 18
# speedup vs baseline: 2.2838x; 1.0011x over previous
# Neural CDE (RK4 3/8-rule scan) Trainium2 Bass kernel.
# Self-contained: hardcodes shapes/sharding; callable as kernel(**inputs).
import sys

sys.path.insert(0, "/opt/trn_rl_repo")
import numpy as np

B, T, IN, HID, HH, OUT = 1024, 128, 16, 64, 64, 10
NCORES = 8
BL = B // NCORES  # batch per core
STEPS = T - 1

_cache = {}


def _get_mult_scan():
    """Custom DVE op: out[k] = cumsum(in0*in1) along free dim (fp32 state)."""
    import concourse.dve_ops as dve_ops
    from concourse.dve_ops import DveOp, get_dve_sub_opcode
    from concourse.dve_spec import Spec, Src0, Src1, scan, AluOp, lower
    from concourse.dve_uop import DveOpSpec

    name = "MULT_SCAN_NCDE"
    for o in dve_ops.OPS:
        if o.name == name:
            return o
    spec = Spec(
        body=scan(AluOp.ADD, Src0 * Src1),
        reference=lambda in0, in1, s0, s1, imm2: np.cumsum(
            (in0.astype(np.float32) * in1.astype(np.float32)), axis=-1
        ),
    )
    op = DveOp(name, spec, subdim=False, uops_sha={})
    dve_ops.OPS.append(op)
    dve_ops.CUSTOM_DVE_SPECS[name] = spec
    dve_ops._SUB_OPCODE_FOR_NAME[name] = (
        dve_ops._CUSTOM_DVE_ROW_BASE + len(dve_ops.OPS) - 1
    )
    for ver in ("v3", "v4"):
        try:
            ds = DveOpSpec(
                name=name,
                opcode=get_dve_sub_opcode(name),
                uops=lower(spec, ver=ver),
                rd1_en=True,
            )
            op.uops_sha[ver] = ds.sha(ver)
        except Exception as e:
            print("mult_scan sha", ver, "failed:", e)
    return op


def _build(steps=STEPS, use_f32r=True, half_dt="float16"):
    import concourse.bacc as bacc
    import concourse.mybir as mybir
    from concourse.tile import TileContext

    fp32 = mybir.dt.float32
    f32r = mybir.dt.float32r if use_f32r else fp32
    fp16 = getattr(mybir.dt, half_dt)
    AF = mybir.ActivationFunctionType
    OP = mybir.AluOpType
    AX = mybir.AxisListType

    mscan = _get_mult_scan()
    nc = bacc.Bacc("TRN2", target_bir_lowering=False, debug=False, num_devices=NCORES)

    # ---- DRAM I/O (per-core shards; weights replicated) ----
    d_a0 = nc.dram_tensor("a0", (BL, IN), fp32, kind="ExternalInput")
    d_cb = nc.dram_tensor("cb", (BL, STEPS * IN), fp32, kind="ExternalInput")
    d_cc = nc.dram_tensor("cc", (BL, STEPS * IN), fp32, kind="ExternalInput")
    d_cd = nc.dram_tensor("cd", (BL, STEPS * IN), fp32, kind="ExternalInput")
    d_w1 = nc.dram_tensor("w1aug", (HID + 1, HH), fp16, kind="ExternalInput")
    d_w2 = nc.dram_tensor("w2aug", (HH + 1, HID * IN), fp16, kind="ExternalInput")
    d_wl = nc.dram_tensor("wlaug", (HID + 1, OUT), fp16, kind="ExternalInput")
    d_wi = nc.dram_tensor("wiaug", (IN + 1, HID), fp32, kind="ExternalInput")
    d_id = nc.dram_tensor("ident", (128, 128), fp32, kind="ExternalInput")
    d_idh = nc.dram_tensor("identh", (128, 128), fp16, kind="ExternalInput")
    d_ones = nc.dram_tensor("ones", (1, 128), fp32, kind="ExternalInput")
    d_onesh = nc.dram_tensor("onesh", (1, 128), fp16, kind="ExternalInput")
    d_y = nc.dram_tensor("y", (BL, T * OUT), fp32, kind="ExternalOutput")

    NC2 = STEPS * IN  # 2032
    H2 = HID // 2  # 32 (column-split half of h)

    with TileContext(nc) as tc, nc.allow_low_precision("fp16 f-path; fp32 state"):
        with (
            tc.tile_pool(name="const", bufs=1) as constp,
            tc.tile_pool(name="state", bufs=1) as statep,
            tc.tile_pool(name="big", bufs=1) as bigp,
            tc.tile_pool(name="fpool", bufs=3) as fpool,
            tc.tile_pool(name="hpsum", bufs=1, space="PSUM") as hpsum,
            tc.tile_pool(name="fpsum", bufs=2, space="PSUM") as fpsum,
            tc.tile_pool(name="tpsum", bufs=3, space="PSUM") as tpsum,
            tc.tile_pool(name="ypsum", bufs=1, space="PSUM") as ypsum,
        ):
            # ---- constants ----
            w1 = constp.tile([HID + 1, HH], fp16, tag="w1")
            w2 = constp.tile([HH + 1, HID * IN], fp16, tag="w2")
            wl = constp.tile([HID + 1, OUT], fp16, tag="wl")
            wi = constp.tile([IN + 1, HID], fp32, tag="wi")
            ident = constp.tile([128, 128], fp32, tag="ident")
            identh = constp.tile([128, 128], fp16, tag="identh")
            nc.sync.dma_start(w1[:], d_w1.ap()[:])
            nc.sync.dma_start(w2[:], d_w2.ap()[:])
            nc.sync.dma_start(wl[:], d_wl.ap()[:])
            nc.sync.dma_start(wi[:], d_wi.ap()[:])
            nc.sync.dma_start(ident[:], d_id.ap()[:])
            nc.sync.dma_start(identh[:], d_idh.ap()[:])

            # ---- spline derivative precompute: u, v, w (fp16) ----
            cb = bigp.tile([BL, NC2], fp32, tag="cb")
            cc = bigp.tile([BL, NC2], fp32, tag="cc")
            cd = bigp.tile([BL, NC2], fp32, tag="cd")
            tt0 = bigp.tile([BL, NC2], fp32, tag="tt0")
            uu = bigp.tile([BL, NC2], fp16, tag="uu")
            vv = bigp.tile([BL, NC2], fp16, tag="vv")
            ww = bigp.tile([BL, NC2], fp16, tag="ww")
            db0 = bigp.tile([BL, IN], fp16, tag="db0")
            nc.sync.dma_start(cb[:], d_cb.ap()[:])
            nc.sync.dma_start(cc[:], d_cc.ap()[:])
            nc.sync.dma_start(cd[:], d_cd.ap()[:])
            stt = nc.vector.scalar_tensor_tensor
            # u = b + c + d ; v = b + c/3 + d/9 ; w = b + 2c/3 + 4d/9
            stt(tt0[:], cc[:], 1.0, cb[:], OP.mult, OP.add)
            stt(uu[:], cd[:], 1.0, tt0[:], OP.mult, OP.add)
            stt(tt0[:], cc[:], 1.0 / 3.0, cb[:], OP.mult, OP.add)
            stt(vv[:], cd[:], 1.0 / 9.0, tt0[:], OP.mult, OP.add)
            stt(tt0[:], cc[:], 2.0 / 3.0, cb[:], OP.mult, OP.add)
            stt(ww[:], cd[:], 4.0 / 9.0, tt0[:], OP.mult, OP.add)
            nc.vector.tensor_copy(db0[:], cb[:, 0:IN])

            # ---- state tiles ----
            zT = statep.tile([HID + 1, BL], fp32, tag="zT")  # master state (T-layout, aug)
            zmid = statep.tile([HID + 1, BL], fp16, tag="zmid")
            zTh = statep.tile([HID + 1, BL], fp16, tag="zTh")
            hT = statep.tile([HH + 1, BL], fp16, tag="hT")
            k1 = statep.tile([HID, BL], fp32, tag="k1")
            k2 = statep.tile([HID, BL], fp32, tag="k2")
            k3 = statep.tile([HID, BL], fp32, tag="k3")
            t1 = statep.tile([HID, BL], fp32, tag="t1")
            t2a = statep.tile([HID, BL], fp32, tag="t2a")
            t2b = statep.tile([HID, BL], fp32, tag="t2b")
            q0 = statep.tile([HID, BL], fp32, tag="q0")
            q1 = statep.tile([HID, BL], fp32, tag="q1")
            q2 = statep.tile([HID, BL], fp32, tag="q2")
            t3c = statep.tile([HID, BL], fp32, tag="t3c")
            zzero = statep.tile([HID, BL], fp32, tag="zzero")
            nc.vector.memset(zzero[:], 0.0)
            ystage = statep.tile([BL, 51 * OUT], fp32, tag="ystage")
            sc0 = statep.tile([BL, 513], fp32, tag="sc0")
            sc1 = statep.tile([BL, 513], fp32, tag="sc1")
            nc.vector.memset(sc0[:, 0:1], 0.0)
            nc.vector.memset(sc1[:, 0:1], 0.0)
            nc.sync.dma_start(zT[HID : HID + 1, :], d_ones.ap()[:])
            nc.sync.dma_start(zmid[HID : HID + 1, :], d_onesh.ap()[:])
            nc.sync.dma_start(zTh[HID : HID + 1, :], d_onesh.ap()[:])
            nc.sync.dma_start(hT[HH : HH + 1, :], d_onesh.ap()[:])

            # ---- z0 = a0 @ W_init + b_init ----
            with tc.tile_pool(name="prol", bufs=1) as prol:
                a0 = prol.tile([BL, IN], fp32, tag="a0")
                a0T = prol.tile([IN + 1, BL], fp32, tag="a0T")
                nc.sync.dma_start(a0[:], d_a0.ap()[:])
                a0Tp = tpsum.tile([IN, BL], fp32, tag="ftp")
                nc.tensor.transpose(a0Tp[:], a0[:], ident[:])
                nc.vector.tensor_copy(a0T[0:IN, :], a0Tp[:])
                nc.sync.dma_start(a0T[IN : IN + 1, :], d_ones.ap()[:])
                z0p = tpsum.tile([HID, BL], fp32, tag="ftp")
                nc.tensor.matmul(z0p[:], wi[:], a0T[:], start=True, stop=True)
                nc.vector.tensor_copy(zT[0:HID, :], z0p[:])
                nc.vector.tensor_copy(zTh[0:HID, :], z0p[:])

            # ---- main RK4 scan ----
            CH = 51  # y steps accumulated per PSUM bank before DMA flush
            yp = None
            pending = []
            for t in range(steps):
                if t % CH == 0:
                    yp = ypsum.tile([BL, CH * OUT], fp32, tag="yp")
                slot = t % CH
                for s in range(4):
                    src = zTh if s == 0 else zmid
                    # mm1 (fp32) + relu (-> f32r for the mm2 weights-side)
                    hp = hpsum.tile([HH, BL], fp32, tag="hp")
                    nc.tensor.matmul(hp[:], w1[:], src[:], start=True, stop=True)
                    if s == 0:
                        # y_t = z_t @ W_lin (+b): PE-idle window, off the chain
                        nc.tensor.matmul(
                            yp[:, slot * OUT : (slot + 1) * OUT], zTh[:], wl[:],
                            start=True, stop=True,
                        )
                    nc.vector.tensor_scalar_max(hT[0:HH, :], hp[:], 0.0)
                    for fn in pending:
                        fn()
                    pending = []
                    # dX slice for this stage
                    if s == 0:
                        dx = db0[:] if t == 0 else uu[:, (t - 1) * IN : t * IN]
                    elif s == 1:
                        dx = vv[:, t * IN : (t + 1) * IN]
                    elif s == 2:
                        dx = ww[:, t * IN : (t + 1) * IN]
                    else:
                        dx = uu[:, t * IN : (t + 1) * IN]
                    dx3 = dx.unsqueeze(1).broadcast_to([BL, H2, IN])
                    F = fpool.tile([BL, HID], fp16, tag="F")
                    ftp = tpsum.tile([HID, BL], fp16, tag="ftp")
                    # column-split halves pipeline mm2->tanh->mult->reduce->transpose
                    for ch in range(2):
                        cols = slice(ch * 512, (ch + 1) * 512)
                        fpp = fpsum.tile([BL, 512], fp32, tag="fp")
                        nc.tensor.matmul(fpp[:], hT[:], w2[:, cols], start=True, stop=True)
                        f_sb = fpool.tile([BL, 512], fp16, tag="f_sb")
                        nc.scalar.activation(f_sb[:], fpp[:], AF.Tanh)
                        sc = sc0 if ch == 0 else sc1
                        nc.vector._custom_dve(
                            mscan, out=sc[:, 1:513], in0=f_sb[:], in1=dx3
                        )
                        scg = sc[:, 1:513].rearrange("p (g i) -> p g i", i=IN)
                        sclo = sc[:, 0:512].rearrange("p (g i) -> p g i", i=IN)
                        Fh = F[:, ch * H2 : (ch + 1) * H2]
                        nc.vector.tensor_tensor(
                            Fh.unsqueeze(2), scg[:, :, IN - 1 : IN], sclo[:, :, 0:1],
                            OP.subtract,
                        )
                    nc.tensor.transpose(ftp[:], F[:], identh[:])
                    # combos (fp32 state updates)
                    if s == 0:
                        # ON-chain: za = z + k1/3
                        stt(zmid[0:HID, :], ftp[:], 1.0 / 3.0, zT[0:HID, :], OP.mult, OP.add)
                        # off-chain prefixes from k1 (emitted after next relu)
                        pending.append(lambda ftp=ftp: (
                            stt(t1[:], ftp[:], -1.0 / 3.0, zT[0:HID, :], OP.mult, OP.add),
                            stt(t2a[:], ftp[:], 1.0, zT[0:HID, :], OP.mult, OP.add),
                            stt(q0[:], ftp[:], 0.125, zzero[:], OP.mult, OP.add),
                        ))
                    elif s == 1:
                        # ON: zb = t1 + k2
                        stt(zmid[0:HID, :], ftp[:], 1.0, t1[:], OP.mult, OP.add)
                        pending.append(lambda ftp=ftp: (
                            stt(t2b[:], ftp[:], -1.0, t2a[:], OP.mult, OP.add),
                            stt(q1[:], ftp[:], 0.375, q0[:], OP.mult, OP.add),
                        ))
                    elif s == 2:
                        # ON: zc = t2b + k3
                        stt(zmid[0:HID, :], ftp[:], 1.0, t2b[:], OP.mult, OP.add)
                        pending.append(lambda ftp=ftp: (
                            stt(q2[:], ftp[:], 0.375, q1[:], OP.mult, OP.add),
                            stt(t3c[:], q2[:], 1.0, zT[0:HID, :], OP.mult, OP.add),
                        ))
                    else:
                        # ON: zn = (z + q2) + k4/8, one fused op
                        stt(zTh[0:HID, :], ftp[:], 0.125, t3c[:], OP.mult, OP.add)
                        pending.append(lambda ftp=ftp: (
                            stt(zT[0:HID, :], ftp[:], 0.125, t3c[:], OP.mult, OP.add),
                        ))
                if slot == CH - 1:
                    c0 = (t // CH) * CH * OUT
                    nc.scalar.activation(ystage[:], yp[:], AF.Copy)
                    nc.sync.dma_start(d_y.ap()[:, c0 : c0 + CH * OUT], ystage[:])
            for fn in pending:
                fn()
            pending = []
            # final y entries: t = steps (last state) plus tail of last chunk
            tg = steps
            if tg % CH == 0:
                yp = ypsum.tile([BL, CH * OUT], fp32, tag="yp")
            slot = tg % CH
            nc.tensor.matmul(
                yp[:, slot * OUT : (slot + 1) * OUT], zTh[:], wl[:],
                start=True, stop=True,
            )
            c0 = (tg // CH) * CH * OUT
            nc.scalar.activation(
                ystage[:, 0 : (slot + 1) * OUT], yp[:, 0 : (slot + 1) * OUT], AF.Copy
            )
            nc.sync.dma_start(
                d_y.ap()[:, c0 : c0 + (slot + 1) * OUT], ystage[:, 0 : (slot + 1) * OUT]
            )

    nc.compile()
    return nc


def _prep_maps(coeff_a, coeff_b, coeff_two_c, coeff_three_d, W_init, b_init, W1, b1, W2, b2, W_lin, b_lin):
    import ml_dtypes

    f32 = np.float32
    f16 = np.float16
    w1aug = np.concatenate([np.asarray(W1, f32), np.asarray(b1, f32)[None]], 0).astype(f16)
    w2aug = np.concatenate([np.asarray(W2, f32), np.asarray(b2, f32)[None]], 0).astype(f16)
    wlaug = np.concatenate([np.asarray(W_lin, f32), np.asarray(b_lin, f32)[None]], 0).astype(f16)
    wiaug = np.concatenate([np.asarray(W_init, f32), np.asarray(b_init, f32)[None]], 0)
    ident = np.eye(128, dtype=f32)
    ca = np.asarray(coeff_a, f32)
    cb = np.asarray(coeff_b, f32)
    cc = np.asarray(coeff_two_c, f32)
    cd = np.asarray(coeff_three_d, f32)
    maps = []
    for c in range(NCORES):
        sl = slice(c * BL, (c + 1) * BL)
        maps.append(
            {
                "a0": np.ascontiguousarray(ca[sl, 0, :]),
                "cb": np.ascontiguousarray(cb[sl].reshape(BL, -1)),
                "cc": np.ascontiguousarray(cc[sl].reshape(BL, -1)),
                "cd": np.ascontiguousarray(cd[sl].reshape(BL, -1)),
                "w1aug": w1aug,
                "w2aug": w2aug,
                "wlaug": wlaug,
                "wiaug": wiaug,
                "ident": ident,
                "identh": np.eye(128, dtype=f16),
                "ones": np.ones((1, 128), f32),
                "onesh": np.ones((1, 128), f16),
            }
        )
    return maps


def _install_ntff_hook():
    """Provide antenv.axon_hooks (missing in this image) so trace=True works."""
    import types, ctypes, contextlib

    try:
        from antenv.axon_hooks import get_axon_ntff_profile_hook  # noqa: F401
        return
    except ImportError:
        pass
    import antenv

    hook = None
    try:
        lib = ctypes.CDLL("/opt/axon/libaxon_pjrt.so")
        if hasattr(lib, "axon_start_nrt_profile"):
            lib.axon_start_nrt_profile.argtypes = [
                ctypes.POINTER(ctypes.c_int64),
                ctypes.c_size_t,
            ]
            lib.axon_start_nrt_profile.restype = ctypes.c_int64
            lib.axon_stop_nrt_profile.argtypes = [ctypes.c_char_p]
            lib.axon_stop_nrt_profile.restype = ctypes.c_int64

            @contextlib.contextmanager
            def _hook(output_dir, device_ids):
                import jax

                jax.devices()
                if device_ids:
                    ids = (ctypes.c_int64 * len(device_ids))(*device_ids)
                    rc = lib.axon_start_nrt_profile(ids, len(device_ids))
                else:
                    rc = lib.axon_start_nrt_profile(None, 0)
                if rc != 0:
                    raise RuntimeError(f"axon_start_nrt_profile rc={rc}")
                try:
                    yield
                finally:
                    n = lib.axon_stop_nrt_profile(str(output_dir).encode())
                    print(f"ntff profile: {n} file(s) written to {output_dir}")

            hook = _hook
    except OSError:
        pass
    mod = types.ModuleType("antenv.axon_hooks")
    mod.get_axon_ntff_profile_hook = lambda: hook
    mod.set_axon_ntff_profile_hook = lambda h: None
    sys.modules["antenv.axon_hooks"] = mod
    antenv.axon_hooks = mod


def kernel(times, coeff_a, coeff_b, coeff_two_c, coeff_three_d, final_index,
           W_init, b_init, W1, b1, W2, b2, W_lin, b_lin, trace=False):
    from concourse import bass_utils

    if trace:
        _install_ntff_hook()

    if "nc" not in _cache:
        _cache["nc"] = _build()
    nc = _cache["nc"]
    maps = _prep_maps(coeff_a, coeff_b, coeff_two_c, coeff_three_d,
                      W_init, b_init, W1, b1, W2, b2, W_lin, b_lin)
    res = bass_utils.run_bass_kernel_spmd(nc, maps, core_ids=list(range(NCORES)), trace=trace)
    _cache["last_res"] = res
    y = np.concatenate(
        [res.results[c]["y"].reshape(BL, T, OUT) for c in range(NCORES)], 0
    )
    return y.astype(np.float32)
